# revision 40
# baseline (speedup 1.0000x reference)
"""CosmosAttention distributed Bass kernel for 8 TRN2 NeuronCores.

Sharding: 8 cores = 2 batches x 4 head-groups (tensor-parallel over heads,
data-parallel over batch). Each core computes QKV + attention for its 4 heads
of its batch, AllGathers attention outputs within its 4-core batch group, and
computes a 512-wide slice of the output projection.

Pipeline (per core), software-pipelined two phases deep so attention never
waits on the rope tail of the same head:
  A:  QK-proj head 0, weight-stationary over d, paced by the xT DMA stream
  B:  V-proj, rope head 0, QK-proj + rope head 1
  C0: attn0, AG0, QK-proj + rope head 2
  C1: attn1, AG1, attn2, AG2, QK-proj + rope head 3
  C3: attn3 (2 chunk-pairs, AllGathered per pair), partial out-proj over
      gathered heads 0-2, final out-proj of head 3, y store

Stationary-weight reuse: matmuls that repeat the previous lhsT are emitted
with InstMatmult.ldweights=False so the PE array skips the weight reload.

PSUM budget (8 banks): tags qk 2x[128,512] + score 2x[128,1024] +
pvden 2x[128,512].
"""

import numpy as np
import ml_dtypes

import concourse.bass as bass
import concourse.mybir as mybir
import concourse.tile as tile
from concourse import bacc
from concourse.bass_utils import run_bass_kernel_spmd

F32 = mybir.dt.float32
BF16 = mybir.dt.bfloat16
BF = ml_dtypes.bfloat16

B, S, DM, H, HD = 2, 2048, 2048, 16, 128
HL = 4           # heads per core
IL = HL * HD     # inner slice per core = 512
ND = DM // 128   # 16 d-tiles
NC = 4           # token chunks of 512
NTB = S // 128   # 16 token blocks
CHUNK = 512
EPS = 1e-6
INV_SQRT_HD = 1.0 / float(np.sqrt(HD))
MULT = mybir.AluOpType.mult
ADD = mybir.AluOpType.add
REUSE_LDW = True

_CACHED = {}


def _build_nc():
    nc = bacc.Bacc("TRN2", target_bir_lowering=False, debug=False, num_devices=8)

    xT = nc.dram_tensor("xT", [DM, S], BF16, kind="ExternalInput")
    wvT = nc.dram_tensor("wvT", [DM, IL], BF16, kind="ExternalInput")
    woT = nc.dram_tensor("woT", [DM, IL], BF16, kind="ExternalInput")
    whq = nc.dram_tensor("whq", [128, HL * ND * 128], BF16, kind="ExternalInput")
    whk = nc.dram_tensor("whk", [128, HL * ND * 128], BF16, kind="ExternalInput")
    caq = nc.dram_tensor("caq", [HD, S], BF16, kind="ExternalInput")
    cbq = nc.dram_tensor("cbq", [HD, S], BF16, kind="ExternalInput")
    cak = nc.dram_tensor("cak", [HD, S], BF16, kind="ExternalInput")
    cbk = nc.dram_tensor("cbk", [HD, S], BF16, kind="ExternalInput")
    yT = nc.dram_tensor("yT", [IL, S], F32, kind="ExternalOutput")

    xT_v = xT.ap().rearrange("(n p) m -> n p m", p=128)
    wv_v = wvT.ap().rearrange("(n p) m -> n p m", p=128)
    wo_v = woT.ap().rearrange("(n p) m -> n p m", p=128)

    def mm(out, lhsT, rhs, start, stop, reuse=False):
        bi = nc.tensor.matmul(out, lhsT, rhs, start=start, stop=stop)
        if reuse and REUSE_LDW:
            bi.ins.ldweights = False
        return bi

    import contextlib
    with tile.TileContext(nc) as tc:
        with contextlib.ExitStack() as _stk:
            def _pool(name, bufs, space="SBUF"):
                return _stk.enter_context(
                    tc.tile_pool(name=name, bufs=bufs, space=space))
            const_pool = _pool("const", 1)
            xt_pool = _pool("xt", ND)
            w_pool = _pool("w", ND)
            wh_pool = _pool("wh", 2)
            cab_pool = _pool("cab", 4)
            qk_pool = _pool("qk", 4)
            v_pool = _pool("v", NTB)
            raw_pool = _pool("raw", 2)
            pre_pool = _pool("pre", 3)
            tmp_pool = _pool("tmp", 2)
            rstd_pool = _pool("rstd", 1)
            sstd_pool = _pool("sstd", 1)
            e_pool = _pool("e", 3)
            ep_pool = _pool("ep", 3)
            small_pool = _pool("small", 2)
            out_pool = _pool("outT", 2)
            ps_pool = _pool("ps", 2, space="PSUM")
            dram_pool = _pool("dram", 1, space="DRAM")

            ones128 = const_pool.tile([128, 128], BF16)
            nc.vector.memset(ones128[:], 1.0)
            eps_tile = const_pool.tile([128, 1], F32)
            nc.vector.memset(eps_tile[:], EPS)

            def ps_qk(name):
                return ps_pool.tile([128, CHUNK], F32, tag="qk", bufs=2,
                                    name=name)

            def ps_pvden(name):
                return ps_pool.tile([128, CHUNK], F32, tag="pvden", bufs=2,
                                    name=name)

            def ps_score(name):
                return ps_pool.tile([128, 2 * CHUNK], F32, tag="score",
                                    bufs=2, name=name)

            # ---------------- input DMAs (sync queue, ordered) -----------
            wh_tiles = {}

            def load_wh(h):
                for part, src in (("q", whq), ("k", whk)):
                    t = wh_pool.tile([128, ND * 128], BF16, tag="wh",
                                     name=f"wh_{part}{h}")
                    nc.sync.dma_start(
                        t[:], src.ap()[:, h * ND * 128:(h + 1) * ND * 128])
                    wh_tiles[(part, h)] = t

            # whq0 first, then xt[0], then whk0 so the first q-matmul can
            # start as early as possible
            t = wh_pool.tile([128, ND * 128], BF16, tag="wh", name="wh_q0")
            nc.sync.dma_start(t[:, 0:4 * 128], whq.ap()[:, 0:4 * 128])
            nc.sync.dma_start(t[:, 4 * 128:ND * 128],
                              whq.ap()[:, 4 * 128:ND * 128])
            wh_tiles[("q", 0)] = t
            xt = []
            t = xt_pool.tile([128, S], BF16, tag="xt", name="xt0")
            nc.sync.dma_start(t[:], xT_v[0])
            xt.append(t)
            t = wh_pool.tile([128, ND * 128], BF16, tag="wh", name="wh_k0")
            nc.sync.dma_start(t[:], whk.ap()[:, 0:ND * 128])
            wh_tiles[("k", 0)] = t
            for d in range(1, ND):
                t = xt_pool.tile([128, S], BF16, tag="xt", name=f"xt{d}")
                nc.sync.dma_start(t[:], xT_v[d])
                xt.append(t)
            wv_tiles = []
            for d in range(ND):
                t = w_pool.tile([128, IL], BF16, tag="w", name=f"wv{d}")
                nc.sync.dma_start(t[:], wv_v[d])
                wv_tiles.append(t)
            ca_q = cab_pool.tile([HD, S], BF16, tag="cab")
            cb_q = cab_pool.tile([HD, S], BF16, tag="cab")
            ca_k = cab_pool.tile([HD, S], BF16, tag="cab")
            cb_k = cab_pool.tile([HD, S], BF16, tag="cab")
            nc.sync.dma_start(ca_q[:], caq.ap())
            nc.sync.dma_start(cb_q[:], cbq.ap())
            nc.sync.dma_start(ca_k[:], cak.ap())
            nc.sync.dma_start(cb_k[:], cbk.ap())
            cab = {"q": (ca_q, cb_q), "k": (ca_k, cb_k)}
            load_wh(1)

            qk_tiles = {}
            raw_tiles = {}
            v_sb = [None] * NTB

            # ---------------- helpers ------------------------------------
            def emit_qkproj(h, part):
                """Projection for one head/part: two chunk-pair passes,
                lhsT reused across the pair."""
                wh = wh_tiles[(part, h)]
                raw = raw_pool.tile([128, S], BF16, tag="raw",
                                    name=f"raw_{part}{h}")
                for cp, mk in ((0, ps_qk), (2, ps_pvden)):
                    acc0 = mk(f"ac_{part}{h}_{cp}")
                    acc1 = mk(f"ac_{part}{h}_{cp + 1}")
                    for d in range(ND):
                        lhsT = wh[:, d * 128:(d + 1) * 128]
                        mm(acc0[:], lhsT,
                           xt[d][:, cp * CHUNK:(cp + 1) * CHUNK],
                           start=(d == 0), stop=(d == ND - 1))
                        mm(acc1[:], lhsT,
                           xt[d][:, (cp + 1) * CHUNK:(cp + 2) * CHUNK],
                           start=(d == 0), stop=(d == ND - 1), reuse=True)
                    with tc.high_priority(offset=150):
                        nc.vector.tensor_copy(
                            raw[:, cp * CHUNK:(cp + 1) * CHUNK], acc0[:])
                        nc.vector.tensor_copy(
                            raw[:, (cp + 1) * CHUNK:(cp + 2) * CHUNK],
                            acc1[:])
                raw_tiles[(part, h)] = raw

            rope_pre = {}
            rstd_tiles = {}

            def emit_rope_pre(h, part):
                """Early rope work (DVE square + swap DMA) emitted right
                after the projection so it overlaps the next unit's
                matmuls."""
                raw = raw_tiles[(part, h)]
                sq = pre_pool.tile([128, S], BF16, tag="pre",
                                   name=f"sq_{part}{h}")
                with tc.high_priority(offset=150):
                    nc.vector.tensor_tensor(sq[:], raw[:], raw[:], op=MULT)
                swap = pre_pool.tile([128, S], BF16, tag="pre",
                                     name=f"swap_{part}{h}")
                nc.sync.dma_start(swap[0:64, :], raw[64:128, :])
                nc.sync.dma_start(swap[64:128, :], raw[0:64, :])
                rope_pre[(part, h)] = (sq, swap)

            def emit_rms(h, part):
                """rms scale: ss matmuls + sqrt (scalar) + reciprocal."""
                sq, _ = rope_pre[(part, h)]
                rstd = rstd_pool.tile([128, S], F32, tag="rstd",
                                      name=f"rstd_{part}{h}")
                for half in range(2):
                    st = ps_score(f"ss_{part}{h}_{half}")
                    for i in range(2):
                        c = 2 * half + i
                        mm(st[:, i * CHUNK:(i + 1) * CHUNK], ones128[:],
                           sq[:, c * CHUNK:(c + 1) * CHUNK],
                           start=True, stop=True, reuse=(c > 0))
                    sstd = sstd_pool.tile([128, 2 * CHUNK], F32, tag="sstd",
                                          name=f"sstd_{part}{h}_{half}")
                    nc.scalar.activation(
                        sstd[:], st[:], mybir.ActivationFunctionType.Sqrt,
                        bias=eps_tile[:], scale=1.0 / HD)
                    with tc.high_priority(offset=150):
                        nc.vector.reciprocal_approx_fast(
                            rstd[:, half * 2 * CHUNK:(half + 1) * 2 * CHUNK],
                            sstd[:])
                rstd_tiles[(part, h)] = rstd

            def emit_rope_final(h, part):
                """rope rotation + rms scale, halves chunked so the first
                1024 columns of the destination land early."""
                raw = raw_tiles[(part, h)]
                ca_t, cb_t = cab[part]
                _, swap = rope_pre[(part, h)]
                rstd = rstd_tiles[(part, h)]
                r1 = tmp_pool.tile([128, S], BF16, tag="tmp",
                                   name=f"r1_{part}{h}")
                r2 = tmp_pool.tile([128, S], BF16, tag="tmp",
                                   name=f"r2_{part}{h}")
                dst = qk_pool.tile([128, S], BF16, tag="qk",
                                   name=f"qk_{part}{h}")
                with tc.high_priority(offset=150):
                    nc.vector.tensor_tensor(r1[:], raw[:], ca_t[:], op=MULT)
                    nc.vector.tensor_tensor(r2[:], swap[:], cb_t[:], op=MULT)
                    for half in range(2):
                        hs = slice(half * 2 * CHUNK, (half + 1) * 2 * CHUNK)
                        nc.vector.tensor_tensor(r1[:, hs], r1[:, hs],
                                                r2[:, hs], op=ADD)
                        nc.vector.tensor_tensor(dst[:, hs], r1[:, hs],
                                                rstd[:, hs], op=MULT)
                qk_tiles[(part, h)] = dst

            def emit_attn_pair(h, pair, outT_h):
                """Attention for head h, query chunks (2*pair, 2*pair+1).
                kh[tb] / v[tb] stationary reused across the chunk pair;
                software-pipelined one tb so PV never waits on exp."""
                hsl = slice(h * 128, (h + 1) * 128)
                sa = slice(2 * pair * CHUNK, (2 * pair + 1) * CHUNK)
                sb = slice((2 * pair + 1) * CHUNK, (2 * pair + 2) * CHUNK)
                qh = qk_tiles[("q", h)]
                kh = qk_tiles[("k", h)]
                pv_a = ps_pvden(f"pva{h}_{pair}")
                pv_b = ps_pvden(f"pvb{h}_{pair}")
                den_a = ps_qk(f"dena{h}_{pair}")
                den_b = ps_qk(f"denb{h}_{pair}")
                prev = None
                e_hold = None
                ep_hold = None
                for tb in range(NTB):
                    st = ps_score(f"s{h}_{pair}_{tb}")
                    khs = kh[:, tb * 128:(tb + 1) * 128]
                    mm(st[:, 0:CHUNK], khs, qh[:, sa], start=True, stop=True)
                    mm(st[:, CHUNK:2 * CHUNK], khs, qh[:, sb],
                       start=True, stop=True, reuse=True)
                    e = e_pool.tile([128, 2 * CHUNK], BF16, tag="e", bufs=3)
                    nc.scalar.activation(e[:], st[:],
                                         mybir.ActivationFunctionType.Exp,
                                         bias=0.0, scale=INV_SQRT_HD)
                    if prev is not None:
                        ptb, pe = prev
                        vt = v_sb[ptb][:, hsl]
                        mm(pv_a[:], vt, pe[:, 0:CHUNK],
                           start=(ptb == 0), stop=False)
                        mm(pv_b[:], vt, pe[:, CHUNK:2 * CHUNK],
                           start=(ptb == 0), stop=False, reuse=True)
                    prev = (tb, e)
                    if tb % 2 == 0:
                        e_hold = e
                    else:
                        ep = ep_pool.tile([128, 2 * CHUNK], BF16, tag="ep",
                                          bufs=3)
                        nc.vector.tensor_tensor(ep[:], e_hold[:], e[:],
                                                op=ADD)
                        mm(den_a[:], ones128[:], ep[:, 0:CHUNK],
                           start=(tb == 1), stop=(tb == NTB - 1))
                        mm(den_b[:], ones128[:], ep[:, CHUNK:2 * CHUNK],
                           start=(tb == 1), stop=(tb == NTB - 1),
                           reuse=True)
                ptb, pe = prev
                vt = v_sb[ptb][:, hsl]
                mm(pv_a[:], vt, pe[:, 0:CHUNK], start=False, stop=True)
                mm(pv_b[:], vt, pe[:, CHUNK:2 * CHUNK], start=False,
                   stop=True, reuse=True)
                for pv, den, ssl in ((pv_a, den_a, sa), (pv_b, den_b, sb)):
                    rden = small_pool.tile([128, CHUNK], F32, tag="rden")
                    nc.vector.reciprocal_approx_fast(rden[:], den[:])
                    nc.vector.tensor_tensor(outT_h[:, ssl], pv[:], rden[:],
                                            op=MULT)

            def emit_ag(h, outT_h):
                ag_in = dram_pool.tile([128, S], BF16, tag="agin",
                                       bufs=HL - 1, name=f"agin{h}")
                nc.gpsimd.dma_start(ag_in[:], outT_h[:])
                ag_out = dram_pool.tile([IL, S], BF16, tag="agout",
                                        bufs=HL - 1, name=f"agout{h}")
                nc.gpsimd.collective_compute(
                    "AllGather",
                    mybir.AluOpType.bypass,
                    replica_groups=[[0, 1, 2, 3], [4, 5, 6, 7]],
                    ins=[ag_in.opt()],
                    outs=[ag_out.opt()],
                )
                ag_outs.append(ag_out)

            # ---------------- Phase A: QK proj head 0 (paced) -----------
            a_accs = {}
            ss0 = ps_score("aq01")
            ss1 = ps_score("aq23")
            a_accs[("q", 0)] = ss0[:, 0:CHUNK]
            a_accs[("q", 1)] = ss0[:, CHUNK:2 * CHUNK]
            a_accs[("q", 2)] = ss1[:, 0:CHUNK]
            a_accs[("q", 3)] = ss1[:, CHUNK:2 * CHUNK]
            a_accs[("k", 0)] = ps_qk("ak0")[:]
            a_accs[("k", 1)] = ps_qk("ak1")[:]
            a_accs[("k", 2)] = ps_pvden("ak2")[:]
            a_accs[("k", 3)] = ps_pvden("ak3")[:]
            for d in range(ND):
                for part in ("q", "k"):
                    lhsT = wh_tiles[(part, 0)][:, d * 128:(d + 1) * 128]
                    for c in range(NC):
                        csl = slice(c * CHUNK, (c + 1) * CHUNK)
                        mm(a_accs[(part, c)], lhsT, xt[d][:, csl],
                           start=(d == 0), stop=(d == ND - 1), reuse=(c > 0))
            for part in ("k", "q"):
                raw = raw_pool.tile([128, S], BF16, tag="raw",
                                    name=f"raw_{part}0")
                for c in range(NC):
                    csl = slice(c * CHUNK, (c + 1) * CHUNK)
                    nc.vector.tensor_copy(raw[:, csl], a_accs[(part, c)])
                raw_tiles[(part, 0)] = raw
                emit_rope_pre(0, part)

            # ---------------- Phase B: V proj, rope0, QK1 ----------------
            load_wh(2)
            for tb in range(NTB):
                tsl = slice(tb * 128, (tb + 1) * 128)
                ps = ps_qk(f"vps{tb}") if tb % 2 == 0 else ps_pvden(f"vps{tb}")
                for d in range(ND):
                    nc.tensor.matmul(ps[:], xt[d][:, tsl], wv_tiles[d][:],
                                     start=(d == 0), stop=(d == ND - 1))
                vt = v_pool.tile([128, IL], BF16, tag="v", name=f"v{tb}")
                nc.vector.tensor_copy(vt[:], ps[:])
                v_sb[tb] = vt
                if tb == 5:
                    emit_rms(0, "q")
                    emit_rope_final(0, "q")
                if tb == 9:
                    emit_rms(0, "k")
                    emit_rope_final(0, "k")
            # wo prefetch through freed wv slots (gpsimd queue)
            wo_tiles = []
            for d in range(ND):
                t = w_pool.tile([128, IL], BF16, tag="w", name=f"wo{d}")
                nc.gpsimd.dma_start(t[:], wo_v[d])
                wo_tiles.append(t)
            emit_qkproj(1, "q")
            emit_rope_pre(1, "q")
            emit_qkproj(1, "k")
            emit_rms(1, "q")
            emit_rope_pre(1, "k")
            emit_rope_final(1, "q")
            emit_rms(1, "k")
            emit_rope_final(1, "k")

            ag_outs = []

            # ---------------- Phase C0: attn0, AG0, QK2 ------------------
            load_wh(3)
            outT0 = out_pool.tile([128, S], BF16, tag="outT", name="outT0")
            emit_attn_pair(0, 0, outT0)
            emit_attn_pair(0, 1, outT0)
            emit_ag(0, outT0)
            emit_qkproj(2, "q")
            emit_rope_pre(2, "q")
            emit_qkproj(2, "k")
            emit_rms(2, "q")
            emit_rope_pre(2, "k")
            emit_rope_final(2, "q")
            emit_rms(2, "k")
            emit_rope_final(2, "k")

            # ---------------- Phase C1: attn1, QK3, attn2 ----------------
            # attn2 runs after QK3 so head-3's rope tail hides under it.
            # gathered head 0 -> SBUF (xt slots 0-3; the loads fire once
            # the QK3 matmuls release those slots)
            ag_sb = [None] * 12
            for j in range(4):
                gv = ag_outs[0].rearrange("(n p) m -> n p m", p=128)
                t = xt_pool.tile([128, S], BF16, tag="xt", name=f"agsb0_{j}")
                nc.sync.dma_start(t[:], gv[j])
                ag_sb[j] = t

            outT1 = out_pool.tile([128, S], BF16, tag="outT", name="outT1")
            ag1_outs = []
            emit_attn_pair(1, 0, outT1)
            # first half AG right after pair 0
            csl = slice(0, 2 * CHUNK)
            ag_in = dram_pool.tile([128, 2 * CHUNK], BF16, tag="agin1",
                                   bufs=2, name="agin1_0")
            nc.gpsimd.dma_start(ag_in[:], outT1[:, csl])
            ag_out = dram_pool.tile([IL, 2 * CHUNK], BF16, tag="agout1",
                                    bufs=2, name="agout1_0")
            nc.gpsimd.collective_compute(
                "AllGather", mybir.AluOpType.bypass,
                replica_groups=[[0, 1, 2, 3], [4, 5, 6, 7]],
                ins=[ag_in.opt()], outs=[ag_out.opt()])
            ag1_outs.append(ag_out)
            emit_attn_pair(1, 1, outT1)
            csl = slice(2 * CHUNK, 4 * CHUNK)
            ag_in = dram_pool.tile([128, 2 * CHUNK], BF16, tag="agin1",
                                   bufs=2, name="agin1_1")
            nc.gpsimd.dma_start(ag_in[:], outT1[:, csl])
            ag_out = dram_pool.tile([IL, 2 * CHUNK], BF16, tag="agout1",
                                    bufs=2, name="agout1_1")
            nc.gpsimd.collective_compute(
                "AllGather", mybir.AluOpType.bypass,
                replica_groups=[[0, 1, 2, 3], [4, 5, 6, 7]],
                ins=[ag_in.opt()], outs=[ag_out.opt()])
            ag1_outs.append(ag_out)

            # QK3 (k first), rope under attn2's cover
            emit_qkproj(3, "k")
            emit_rope_pre(3, "k")
            emit_qkproj(3, "q")
            emit_rms(3, "k")
            emit_rope_pre(3, "q")
            emit_rope_final(3, "k")
            emit_rms(3, "q")
            emit_rope_final(3, "q")
            # gathered head 1 -> SBUF (xt slots 4-7, freed by QK3)
            g1v = [ag1_outs[half].rearrange("(n p) m -> n p m", p=128)
                   for half in range(2)]
            for j in range(4):
                t = xt_pool.tile([128, S], BF16, tag="xt", name=f"agsb1_{j}")
                nc.sync.dma_start(t[:, 0:2 * CHUNK], g1v[0][j])
                nc.sync.dma_start(t[:, 2 * CHUNK:4 * CHUNK], g1v[1][j])
                ag_sb[4 + j] = t

            outT2 = out_pool.tile([128, S], BF16, tag="outT", name="outT2")
            ag2_outs = []
            emit_attn_pair(2, 0, outT2)
            csl = slice(0, 2 * CHUNK)
            ag_in = dram_pool.tile([128, 2 * CHUNK], BF16, tag="agin2",
                                   bufs=2, name="agin2_0")
            nc.gpsimd.dma_start(ag_in[:], outT2[:, csl])
            ag_out = dram_pool.tile([IL, 2 * CHUNK], BF16, tag="agout2",
                                    bufs=2, name="agout2_0")
            nc.gpsimd.collective_compute(
                "AllGather", mybir.AluOpType.bypass,
                replica_groups=[[0, 1, 2, 3], [4, 5, 6, 7]],
                ins=[ag_in.opt()], outs=[ag_out.opt()])
            ag2_outs.append(ag_out)
            emit_attn_pair(2, 1, outT2)
            csl = slice(2 * CHUNK, 4 * CHUNK)
            ag_in = dram_pool.tile([128, 2 * CHUNK], BF16, tag="agin2",
                                   bufs=2, name="agin2_1")
            nc.gpsimd.dma_start(ag_in[:], outT2[:, csl])
            ag_out = dram_pool.tile([IL, 2 * CHUNK], BF16, tag="agout2",
                                    bufs=2, name="agout2_1")
            nc.gpsimd.collective_compute(
                "AllGather", mybir.AluOpType.bypass,
                replica_groups=[[0, 1, 2, 3], [4, 5, 6, 7]],
                ins=[ag_in.opt()], outs=[ag_out.opt()])
            ag2_outs.append(ag_out)

            g2v = [ag2_outs[half].rearrange("(n p) m -> n p m", p=128)
                   for half in range(2)]
            for j in range(4):
                t = xt_pool.tile([128, S], BF16, tag="xt", name=f"agsb2_{j}")
                nc.sync.dma_start(t[:, 0:2 * CHUNK], g2v[0][j])
                nc.sync.dma_start(t[:, 2 * CHUNK:4 * CHUNK], g2v[1][j])
                ag_sb[8 + j] = t
            yacc = []
            for mt in range(HL):
                t = xt_pool.tile([128, S], BF16, tag="xt", name=f"yacc{mt}")
                yacc.append(t)

            # ---------------- Phase C3 -----------------------------------
            h = HL - 1
            outT3 = out_pool.tile([128, S], BF16, tag="outT", name="outT3")
            ag3_outs = []

            def emit_ag3(half):
                csl = slice(half * 2 * CHUNK, (half + 1) * 2 * CHUNK)
                ag_in = dram_pool.tile([128, 2 * CHUNK], BF16, tag="agin3",
                                       bufs=2, name=f"agin3_{half}")
                nc.gpsimd.dma_start(ag_in[:], outT3[:, csl])
                ag_out = dram_pool.tile([IL, 2 * CHUNK], BF16, tag="agout3",
                                        bufs=2, name=f"agout3_{half}")
                nc.gpsimd.collective_compute(
                    "AllGather",
                    mybir.AluOpType.bypass,
                    replica_groups=[[0, 1, 2, 3], [4, 5, 6, 7]],
                    ins=[ag_in.opt()],
                    outs=[ag_out.opt()],
                )
                ag3_outs.append(ag_out)

            emit_attn_pair(h, 0, outT3)
            emit_ag3(0)
            emit_attn_pair(h, 1, outT3)
            emit_ag3(1)

            # partial out-proj over gathered heads 0-2 -> yacc (bf16)
            for mt in range(HL):
                msl = slice(mt * 128, (mt + 1) * 128)
                st = ps_score(f"yp{mt}_hi")
                yps = [ps_qk(f"yp{mt}_0"), ps_pvden(f"yp{mt}_1"),
                       st[:, 0:CHUNK], st[:, CHUNK:2 * CHUNK]]
                for gj in range(12):
                    lhsT = wo_tiles[gj][:, msl]
                    for c in range(NC):
                        csl = slice(c * CHUNK, (c + 1) * CHUNK)
                        mm(yps[c][:], lhsT, ag_sb[gj][:, csl],
                           start=(gj == 0), stop=(gj == 11), reuse=(c > 0))
                for c in range(NC):
                    csl = slice(c * CHUNK, (c + 1) * CHUNK)
                    nc.vector.tensor_copy(yacc[mt][:, csl], yps[c][:])

            # ---------------- Tail: head-3 out-proj + store --------------
            ag3_sb = []
            for half in range(2):
                gv = ag3_outs[half].rearrange("(n p) m -> n p m", p=128)
                for j in range(4):
                    t = xt_pool.tile([128, 2 * CHUNK], BF16, tag="xt",
                                     name=f"ag3sb{half}_{j}")
                    nc.sync.dma_start(t[:], gv[j])
                    ag3_sb.append(t)

            for mt in range(HL):
                msl = slice(mt * 128, (mt + 1) * 128)
                st = ps_score(f"z{mt}_hi")
                yps = [ps_qk(f"z{mt}_0"), ps_pvden(f"z{mt}_1"),
                       st[:, 0:CHUNK], st[:, CHUNK:2 * CHUNK]]
                for j in range(4):
                    lhsT = wo_tiles[12 + j][:, msl]
                    for c in range(NC):
                        src = ag3_sb[(c // 2) * 4 + j][:, (c % 2) * CHUNK:
                                                       (c % 2 + 1) * CHUNK]
                        mm(yps[c][:], lhsT, src,
                           start=(j == 0), stop=(j == 3), reuse=(c > 0))
                for c in range(NC):
                    csl = slice(c * CHUNK, (c + 1) * CHUNK)
                    ysb = small_pool.tile([128, CHUNK], F32, tag="ysb",
                                          bufs=2)
                    nc.vector.tensor_tensor(ysb[:], yps[c][:],
                                            yacc[mt][:, csl], op=ADD)
                    nc.sync.dma_start(yT.ap()[msl, csl], ysb[:])

    _dedupe_ldweights(nc)
    nc.finalize()
    return nc


# psum-tile name prefixes whose accumulation chains are safe to merge:
# nothing waits on their interior matmuls' semaphore values except
# long-delay buffer-reuse edges (wo prefetch, ag3 staging loads).
_MERGE_PREFIXES = ("ac_", "vps", "aq", "ak", "yp", "z")


def _merge_chain_updates(nc):
    """Move interior accumulation-chain matmul semaphore increments onto the
    chain's stop matmul (one sem-add-imm instead of N serialized sem-incs).
    The engine pays ~26ns per semaphore write; chains are 12-16 matmuls."""
    for f in nc.m.functions:
        for b in f.blocks:
            cur = {}
            chains = []
            for i in b.instructions:
                if not isinstance(i, mybir.InstMatmult):
                    continue
                out = i.outs[0]
                ref = getattr(out, "memref", None)
                if ref is None or not ref.startswith(_MERGE_PREFIXES):
                    continue
                start = bool(i.start_tensor_calc)
                stop = bool(i.stop_tensor_calc)
                if start and stop:
                    continue
                if start:
                    cur[ref] = [i]
                elif ref in cur:
                    cur[ref].append(i)
                    if stop:
                        chains.append(cur.pop(ref))
            for mem in chains:
                stop_i = mem[-1]
                ssi = stop_i.sync_info
                if ssi is None or len(ssi.on_update) != 1:
                    continue
                su = ssi.on_update[0]
                if su.sync_type != "semaphore" or su.update_mode not in (
                        "sem-inc", "sem-add-imm"):
                    continue
                moved = 0
                for i in mem[:-1]:
                    si = i.sync_info
                    if si is None or len(si.on_update) != 1:
                        continue
                    u = si.on_update[0]
                    if (u.sync_type == "semaphore" and u.id == su.id
                            and u.update_mode == "sem-inc"
                            and u.update_value == 1):
                        i.sync_info = mybir.SyncInfo(
                            on_wait=list(si.on_wait), on_update=[])
                        moved += 1
                if moved:
                    ssi.on_update = [mybir.SyncUpdate(
                        sync_type=su.sync_type, id=su.id,
                        ant_name=su.ant_name, update_mode="sem-add-imm",
                        update_value=su.update_value + moved,
                        update_reg=su.update_reg)]
                    stop_i.sync_info = ssi


def _ldw_sig(i):
    ap = i.ins[0]
    try:
        return (ap.memref, ap.offset, str(ap.ap), str(ap.dtype),
                str(i.perf_mode), str(i.tile_position), str(i.tile_size),
                bool(i.is_transpose))
    except Exception:
        return None


def _dedupe_ldweights(nc):
    """Drop InstLdweights that reload the exact weights already resident in
    the PE array (emitted per-matmul by the framework even when consecutive
    matmuls share the stationary operand). Only sync-free repeats are
    removed, so all semaphore waits/updates are preserved."""
    for f in nc.m.functions:
        for b in f.blocks:
            cur = None
            keep = []
            for i in b.instructions:
                if isinstance(i, mybir.InstLdweights):
                    sig = _ldw_sig(i)
                    si = i.sync_info
                    clean = si is None or (len(si.on_wait) == 0
                                           and len(si.on_update) == 0)
                    if sig is not None and sig == cur and clean:
                        continue
                    cur = sig
                elif isinstance(i, mybir.InstMatmult):
                    if i.is_transpose:
                        cur = None
                keep.append(i)
            if len(keep) != len(b.instructions):
                b.instructions = keep


# inner-dim permutation for per-head AllGather order:
# block (g, j) of gathered = rank j's local head g = global inner
# [(4*j + g)*128 : (4*j + g + 1)*128]
_WO_PERM = np.concatenate(
    [np.arange(128) + (4 * j + g) * 128 for g in range(4) for j in range(4)])


def _host_prep(x, rope_emb, w_q, w_k, w_v, w_o, q_norm_w, k_norm_w):
    """Build the 8 per-core input maps."""
    f = rope_emb[:, 0].astype(np.float32)  # [S, 64, 2, 2]

    def coeffs(w):
        ca = np.empty((HD, S), np.float32)
        cb = np.empty((HD, S), np.float32)
        ca[0:64] = f[:, :, 0, 0].T * w[0:64, None]
        ca[64:128] = f[:, :, 1, 1].T * w[64:128, None]
        cb[0:64] = f[:, :, 0, 1].T * w[64:128, None]
        cb[64:128] = f[:, :, 1, 0].T * w[0:64, None]
        return ca.astype(BF), cb.astype(BF)

    caq, cbq = coeffs(q_norm_w.astype(np.float32))
    cak, cbk = coeffs(k_norm_w.astype(np.float32))

    def pack_wh(w_slice):
        # w_slice: [IL, DM], head-major rows. Output [128, HL*ND*128]
        # with value[p, h*2048 + d*128 + c] = w_slice[h*128+c, d*128+p].
        wt = w_slice.T.astype(np.float32)          # [DM, IL]
        out = np.empty((128, HL * ND * 128), np.float32)
        for hh in range(HL):
            blk = wt[:, hh * 128:(hh + 1) * 128]   # [DM, 128]
            blk = blk.reshape(ND, 128, 128).transpose(1, 0, 2)
            out[:, hh * ND * 128:(hh + 1) * ND * 128] = blk.reshape(128, -1)
        return out.astype(BF)

    in_maps = []
    for c in range(8):
        b, hg = c // 4, c % 4
        sl = slice(IL * hg, IL * (hg + 1))
        in_maps.append({
            "xT": np.ascontiguousarray(x[b].T).astype(BF),
            "whq": pack_wh(w_q[sl]),
            "whk": pack_wh(w_k[sl]),
            "wvT": np.ascontiguousarray(w_v[sl].T).astype(BF),
            "woT": np.ascontiguousarray(w_o[sl][:, _WO_PERM].T).astype(BF),
            "caq": caq, "cbq": cbq, "cak": cak, "cbk": cbk,
        })
    return in_maps


def kernel(x, rope_emb, w_q, w_k, w_v, w_o, q_norm_w, k_norm_w, trace=False):
    x = np.asarray(x, dtype=np.float32)
    rope_emb = np.asarray(rope_emb, dtype=np.float32)
    w_q = np.asarray(w_q, dtype=np.float32)
    w_k = np.asarray(w_k, dtype=np.float32)
    w_v = np.asarray(w_v, dtype=np.float32)
    w_o = np.asarray(w_o, dtype=np.float32)
    q_norm_w = np.asarray(q_norm_w, dtype=np.float32)
    k_norm_w = np.asarray(k_norm_w, dtype=np.float32)
    if "nc" not in _CACHED:
        _CACHED["nc"] = _build_nc()
    nc = _CACHED["nc"]
    in_maps = _host_prep(x, rope_emb, w_q, w_k, w_v, w_o, q_norm_w, k_norm_w)
    res = run_bass_kernel_spmd(nc, in_maps, core_ids=list(range(8)),
                               trace=trace)
    _CACHED["last_result"] = res
    y = np.empty((B, S, DM), np.float32)
    for c in range(8):
        b, hg = c // 4, c % 4
        y[b, :, IL * hg:IL * (hg + 1)] = res.results[c]["yT"].T
    return y


# revision 41
# speedup vs baseline: 1.0204x; 1.0204x over previous
"""CosmosAttention distributed Bass kernel for 8 TRN2 NeuronCores.

Sharding: 8 cores = 2 batches x 4 head-groups (tensor-parallel over heads,
data-parallel over batch). Each core computes QKV + attention for its 4 heads
of its batch, AllGathers attention outputs within its 4-core batch group, and
computes a 512-wide slice of the output projection.

Pipeline (per core), software-pipelined two phases deep so attention never
waits on the rope tail of the same head:
  A:  QK-proj head 0, weight-stationary over d, paced by the xT DMA stream
  B:  V-proj, rope head 0, QK-proj + rope head 1
  C0: attn0, AG0, QK-proj + rope head 2
  C1: attn1, AG1, attn2, AG2, QK-proj + rope head 3
  C3: attn3 (2 chunk-pairs, AllGathered per pair), partial out-proj over
      gathered heads 0-2, final out-proj of head 3, y store

Stationary-weight reuse: matmuls that repeat the previous lhsT are emitted
with InstMatmult.ldweights=False so the PE array skips the weight reload.

PSUM budget (8 banks): tags qk 2x[128,512] + score 2x[128,1024] +
pvden 2x[128,512].
"""

import numpy as np
import ml_dtypes

import concourse.bass as bass
import concourse.mybir as mybir
import concourse.tile as tile
from concourse import bacc
from concourse.bass_utils import run_bass_kernel_spmd

F32 = mybir.dt.float32
BF16 = mybir.dt.bfloat16
BF = ml_dtypes.bfloat16

B, S, DM, H, HD = 2, 2048, 2048, 16, 128
HL = 4           # heads per core
IL = HL * HD     # inner slice per core = 512
ND = DM // 128   # 16 d-tiles
NC = 4           # token chunks of 512
NTB = S // 128   # 16 token blocks
CHUNK = 512
EPS = 1e-6
INV_SQRT_HD = 1.0 / float(np.sqrt(HD))
MULT = mybir.AluOpType.mult
ADD = mybir.AluOpType.add
REUSE_LDW = True

_CACHED = {}


def _build_nc():
    nc = bacc.Bacc("TRN2", target_bir_lowering=False, debug=False, num_devices=8)

    xT = nc.dram_tensor("xT", [DM, S], BF16, kind="ExternalInput")
    wvT = nc.dram_tensor("wvT", [DM, IL], BF16, kind="ExternalInput")
    woT = nc.dram_tensor("woT", [DM, IL], BF16, kind="ExternalInput")
    whq = nc.dram_tensor("whq", [128, HL * ND * 128], BF16, kind="ExternalInput")
    whk = nc.dram_tensor("whk", [128, HL * ND * 128], BF16, kind="ExternalInput")
    caq = nc.dram_tensor("caq", [HD, S], BF16, kind="ExternalInput")
    cbq = nc.dram_tensor("cbq", [HD, S], BF16, kind="ExternalInput")
    cak = nc.dram_tensor("cak", [HD, S], BF16, kind="ExternalInput")
    cbk = nc.dram_tensor("cbk", [HD, S], BF16, kind="ExternalInput")
    yT = nc.dram_tensor("yT", [IL, S], BF16, kind="ExternalOutput")

    xT_v = xT.ap().rearrange("(n p) m -> n p m", p=128)
    wv_v = wvT.ap().rearrange("(n p) m -> n p m", p=128)
    wo_v = woT.ap().rearrange("(n p) m -> n p m", p=128)

    def mm(out, lhsT, rhs, start, stop, reuse=False):
        bi = nc.tensor.matmul(out, lhsT, rhs, start=start, stop=stop)
        if reuse and REUSE_LDW:
            bi.ins.ldweights = False
        return bi

    import contextlib
    with tile.TileContext(nc) as tc:
        with contextlib.ExitStack() as _stk:
            def _pool(name, bufs, space="SBUF"):
                return _stk.enter_context(
                    tc.tile_pool(name=name, bufs=bufs, space=space))
            const_pool = _pool("const", 1)
            xt_pool = _pool("xt", ND)
            w_pool = _pool("w", ND)
            wh_pool = _pool("wh", 2)
            cab_pool = _pool("cab", 4)
            qk_pool = _pool("qk", 4)
            v_pool = _pool("v", NTB)
            raw_pool = _pool("raw", 2)
            pre_pool = _pool("pre", 3)
            tmp_pool = _pool("tmp", 2)
            rstd_pool = _pool("rstd", 1)
            sstd_pool = _pool("sstd", 1)
            e_pool = _pool("e", 3)
            ep_pool = _pool("ep", 3)
            small_pool = _pool("small", 2)
            out_pool = _pool("outT", 2)
            ps_pool = _pool("ps", 2, space="PSUM")
            dram_pool = _pool("dram", 1, space="DRAM")

            ones128 = const_pool.tile([128, 128], BF16)
            nc.vector.memset(ones128[:], 1.0)
            eps_tile = const_pool.tile([128, 1], F32)
            nc.vector.memset(eps_tile[:], EPS)

            def ps_qk(name):
                return ps_pool.tile([128, CHUNK], F32, tag="qk", bufs=2,
                                    name=name)

            def ps_pvden(name):
                return ps_pool.tile([128, CHUNK], F32, tag="pvden", bufs=2,
                                    name=name)

            def ps_score(name):
                return ps_pool.tile([128, 2 * CHUNK], F32, tag="score",
                                    bufs=2, name=name)

            # ---------------- input DMAs (sync queue, ordered) -----------
            wh_tiles = {}

            def load_wh(h):
                for part, src in (("q", whq), ("k", whk)):
                    t = wh_pool.tile([128, ND * 128], BF16, tag="wh",
                                     name=f"wh_{part}{h}")
                    nc.sync.dma_start(
                        t[:], src.ap()[:, h * ND * 128:(h + 1) * ND * 128])
                    wh_tiles[(part, h)] = t

            # whq0 first, then xt[0], then whk0 so the first q-matmul can
            # start as early as possible
            t = wh_pool.tile([128, ND * 128], BF16, tag="wh", name="wh_q0")
            nc.sync.dma_start(t[:, 0:4 * 128], whq.ap()[:, 0:4 * 128])
            nc.sync.dma_start(t[:, 4 * 128:ND * 128],
                              whq.ap()[:, 4 * 128:ND * 128])
            wh_tiles[("q", 0)] = t
            xt = []
            t = xt_pool.tile([128, S], BF16, tag="xt", name="xt0")
            nc.sync.dma_start(t[:], xT_v[0])
            xt.append(t)
            t = wh_pool.tile([128, ND * 128], BF16, tag="wh", name="wh_k0")
            nc.sync.dma_start(t[:], whk.ap()[:, 0:ND * 128])
            wh_tiles[("k", 0)] = t
            for d in range(1, ND):
                t = xt_pool.tile([128, S], BF16, tag="xt", name=f"xt{d}")
                nc.sync.dma_start(t[:], xT_v[d])
                xt.append(t)
            wv_tiles = []
            for d in range(ND):
                t = w_pool.tile([128, IL], BF16, tag="w", name=f"wv{d}")
                nc.sync.dma_start(t[:], wv_v[d])
                wv_tiles.append(t)
            ca_q = cab_pool.tile([HD, S], BF16, tag="cab")
            cb_q = cab_pool.tile([HD, S], BF16, tag="cab")
            ca_k = cab_pool.tile([HD, S], BF16, tag="cab")
            cb_k = cab_pool.tile([HD, S], BF16, tag="cab")
            nc.sync.dma_start(ca_q[:], caq.ap())
            nc.sync.dma_start(cb_q[:], cbq.ap())
            nc.sync.dma_start(ca_k[:], cak.ap())
            nc.sync.dma_start(cb_k[:], cbk.ap())
            cab = {"q": (ca_q, cb_q), "k": (ca_k, cb_k)}
            load_wh(1)

            qk_tiles = {}
            raw_tiles = {}
            v_sb = [None] * NTB

            # ---------------- helpers ------------------------------------
            def emit_qkproj(h, part):
                """Projection for one head/part: two chunk-pair passes,
                lhsT reused across the pair."""
                wh = wh_tiles[(part, h)]
                raw = raw_pool.tile([128, S], BF16, tag="raw",
                                    name=f"raw_{part}{h}")
                for cp, mk in ((0, ps_qk), (2, ps_pvden)):
                    acc0 = mk(f"ac_{part}{h}_{cp}")
                    acc1 = mk(f"ac_{part}{h}_{cp + 1}")
                    for d in range(ND):
                        lhsT = wh[:, d * 128:(d + 1) * 128]
                        mm(acc0[:], lhsT,
                           xt[d][:, cp * CHUNK:(cp + 1) * CHUNK],
                           start=(d == 0), stop=(d == ND - 1))
                        mm(acc1[:], lhsT,
                           xt[d][:, (cp + 1) * CHUNK:(cp + 2) * CHUNK],
                           start=(d == 0), stop=(d == ND - 1), reuse=True)
                    with tc.high_priority(offset=150):
                        nc.vector.tensor_copy(
                            raw[:, cp * CHUNK:(cp + 1) * CHUNK], acc0[:])
                        nc.vector.tensor_copy(
                            raw[:, (cp + 1) * CHUNK:(cp + 2) * CHUNK],
                            acc1[:])
                raw_tiles[(part, h)] = raw

            rope_pre = {}
            rstd_tiles = {}

            def emit_rope_pre(h, part):
                """Early rope work (DVE square + swap DMA) emitted right
                after the projection so it overlaps the next unit's
                matmuls."""
                raw = raw_tiles[(part, h)]
                sq = pre_pool.tile([128, S], BF16, tag="pre",
                                   name=f"sq_{part}{h}")
                with tc.high_priority(offset=150):
                    nc.vector.tensor_tensor(sq[:], raw[:], raw[:], op=MULT)
                swap = pre_pool.tile([128, S], BF16, tag="pre",
                                     name=f"swap_{part}{h}")
                nc.sync.dma_start(swap[0:64, :], raw[64:128, :])
                nc.sync.dma_start(swap[64:128, :], raw[0:64, :])
                rope_pre[(part, h)] = (sq, swap)

            def emit_rms(h, part):
                """rms scale: ss matmuls + sqrt (scalar) + reciprocal."""
                sq, _ = rope_pre[(part, h)]
                rstd = rstd_pool.tile([128, S], F32, tag="rstd",
                                      name=f"rstd_{part}{h}")
                for half in range(2):
                    st = ps_score(f"ss_{part}{h}_{half}")
                    for i in range(2):
                        c = 2 * half + i
                        mm(st[:, i * CHUNK:(i + 1) * CHUNK], ones128[:],
                           sq[:, c * CHUNK:(c + 1) * CHUNK],
                           start=True, stop=True, reuse=(c > 0))
                    sstd = sstd_pool.tile([128, 2 * CHUNK], F32, tag="sstd",
                                          name=f"sstd_{part}{h}_{half}")
                    nc.scalar.activation(
                        sstd[:], st[:], mybir.ActivationFunctionType.Sqrt,
                        bias=eps_tile[:], scale=1.0 / HD)
                    with tc.high_priority(offset=150):
                        nc.vector.reciprocal_approx_fast(
                            rstd[:, half * 2 * CHUNK:(half + 1) * 2 * CHUNK],
                            sstd[:])
                rstd_tiles[(part, h)] = rstd

            def emit_rope_final(h, part):
                """rope rotation + rms scale, halves chunked so the first
                1024 columns of the destination land early."""
                raw = raw_tiles[(part, h)]
                ca_t, cb_t = cab[part]
                _, swap = rope_pre[(part, h)]
                rstd = rstd_tiles[(part, h)]
                r1 = tmp_pool.tile([128, S], BF16, tag="tmp",
                                   name=f"r1_{part}{h}")
                r2 = tmp_pool.tile([128, S], BF16, tag="tmp",
                                   name=f"r2_{part}{h}")
                dst = qk_pool.tile([128, S], BF16, tag="qk",
                                   name=f"qk_{part}{h}")
                with tc.high_priority(offset=150):
                    nc.vector.tensor_tensor(r1[:], raw[:], ca_t[:], op=MULT)
                    nc.vector.tensor_tensor(r2[:], swap[:], cb_t[:], op=MULT)
                    for half in range(2):
                        hs = slice(half * 2 * CHUNK, (half + 1) * 2 * CHUNK)
                        nc.vector.tensor_tensor(r1[:, hs], r1[:, hs],
                                                r2[:, hs], op=ADD)
                        nc.vector.tensor_tensor(dst[:, hs], r1[:, hs],
                                                rstd[:, hs], op=MULT)
                qk_tiles[(part, h)] = dst

            def emit_attn_pair(h, pair, outT_h):
                """Attention for head h, query chunks (2*pair, 2*pair+1).
                kh[tb] / v[tb] stationary reused across the chunk pair;
                software-pipelined one tb so PV never waits on exp."""
                hsl = slice(h * 128, (h + 1) * 128)
                sa = slice(2 * pair * CHUNK, (2 * pair + 1) * CHUNK)
                sb = slice((2 * pair + 1) * CHUNK, (2 * pair + 2) * CHUNK)
                qh = qk_tiles[("q", h)]
                kh = qk_tiles[("k", h)]
                pv_a = ps_pvden(f"pva{h}_{pair}")
                pv_b = ps_pvden(f"pvb{h}_{pair}")
                den_a = ps_qk(f"dena{h}_{pair}")
                den_b = ps_qk(f"denb{h}_{pair}")
                prev = None
                e_hold = None
                ep_hold = None
                for tb in range(NTB):
                    st = ps_score(f"s{h}_{pair}_{tb}")
                    khs = kh[:, tb * 128:(tb + 1) * 128]
                    mm(st[:, 0:CHUNK], khs, qh[:, sa], start=True, stop=True)
                    mm(st[:, CHUNK:2 * CHUNK], khs, qh[:, sb],
                       start=True, stop=True, reuse=True)
                    e = e_pool.tile([128, 2 * CHUNK], BF16, tag="e", bufs=3)
                    nc.scalar.activation(e[:], st[:],
                                         mybir.ActivationFunctionType.Exp,
                                         bias=0.0, scale=INV_SQRT_HD)
                    if prev is not None:
                        ptb, pe = prev
                        vt = v_sb[ptb][:, hsl]
                        mm(pv_a[:], vt, pe[:, 0:CHUNK],
                           start=(ptb == 0), stop=False)
                        mm(pv_b[:], vt, pe[:, CHUNK:2 * CHUNK],
                           start=(ptb == 0), stop=False, reuse=True)
                    prev = (tb, e)
                    if tb % 2 == 0:
                        e_hold = e
                    else:
                        ep = ep_pool.tile([128, 2 * CHUNK], BF16, tag="ep",
                                          bufs=3)
                        nc.vector.tensor_tensor(ep[:], e_hold[:], e[:],
                                                op=ADD)
                        mm(den_a[:], ones128[:], ep[:, 0:CHUNK],
                           start=(tb == 1), stop=(tb == NTB - 1))
                        mm(den_b[:], ones128[:], ep[:, CHUNK:2 * CHUNK],
                           start=(tb == 1), stop=(tb == NTB - 1),
                           reuse=True)
                ptb, pe = prev
                vt = v_sb[ptb][:, hsl]
                mm(pv_a[:], vt, pe[:, 0:CHUNK], start=False, stop=True)
                mm(pv_b[:], vt, pe[:, CHUNK:2 * CHUNK], start=False,
                   stop=True, reuse=True)
                for pv, den, ssl in ((pv_a, den_a, sa), (pv_b, den_b, sb)):
                    rden = small_pool.tile([128, CHUNK], F32, tag="rden")
                    nc.vector.reciprocal_approx_fast(rden[:], den[:])
                    nc.vector.tensor_tensor(outT_h[:, ssl], pv[:], rden[:],
                                            op=MULT)

            def emit_ag(h, outT_h):
                ag_in = dram_pool.tile([128, S], BF16, tag="agin",
                                       bufs=HL - 1, name=f"agin{h}")
                nc.gpsimd.dma_start(ag_in[:], outT_h[:])
                ag_out = dram_pool.tile([IL, S], BF16, tag="agout",
                                        bufs=HL - 1, name=f"agout{h}")
                nc.gpsimd.collective_compute(
                    "AllGather",
                    mybir.AluOpType.bypass,
                    replica_groups=[[0, 1, 2, 3], [4, 5, 6, 7]],
                    ins=[ag_in.opt()],
                    outs=[ag_out.opt()],
                )
                ag_outs.append(ag_out)

            # ---------------- Phase A: QK proj head 0 (paced) -----------
            a_accs = {}
            ss0 = ps_score("aq01")
            ss1 = ps_score("aq23")
            a_accs[("q", 0)] = ss0[:, 0:CHUNK]
            a_accs[("q", 1)] = ss0[:, CHUNK:2 * CHUNK]
            a_accs[("q", 2)] = ss1[:, 0:CHUNK]
            a_accs[("q", 3)] = ss1[:, CHUNK:2 * CHUNK]
            a_accs[("k", 0)] = ps_qk("ak0")[:]
            a_accs[("k", 1)] = ps_qk("ak1")[:]
            a_accs[("k", 2)] = ps_pvden("ak2")[:]
            a_accs[("k", 3)] = ps_pvden("ak3")[:]
            for d in range(ND):
                for part in ("q", "k"):
                    lhsT = wh_tiles[(part, 0)][:, d * 128:(d + 1) * 128]
                    for c in range(NC):
                        csl = slice(c * CHUNK, (c + 1) * CHUNK)
                        mm(a_accs[(part, c)], lhsT, xt[d][:, csl],
                           start=(d == 0), stop=(d == ND - 1), reuse=(c > 0))
            for part in ("k", "q"):
                raw = raw_pool.tile([128, S], BF16, tag="raw",
                                    name=f"raw_{part}0")
                for c in range(NC):
                    csl = slice(c * CHUNK, (c + 1) * CHUNK)
                    nc.vector.tensor_copy(raw[:, csl], a_accs[(part, c)])
                raw_tiles[(part, 0)] = raw
                emit_rope_pre(0, part)

            # ---------------- Phase B: V proj, rope0, QK1 ----------------
            load_wh(2)
            for tb in range(NTB):
                tsl = slice(tb * 128, (tb + 1) * 128)
                ps = ps_qk(f"vps{tb}") if tb % 2 == 0 else ps_pvden(f"vps{tb}")
                for d in range(ND):
                    nc.tensor.matmul(ps[:], xt[d][:, tsl], wv_tiles[d][:],
                                     start=(d == 0), stop=(d == ND - 1))
                vt = v_pool.tile([128, IL], BF16, tag="v", name=f"v{tb}")
                nc.vector.tensor_copy(vt[:], ps[:])
                v_sb[tb] = vt
                if tb == 5:
                    emit_rms(0, "q")
                    emit_rope_final(0, "q")
                if tb == 9:
                    emit_rms(0, "k")
                    emit_rope_final(0, "k")
            # wo prefetch through freed wv slots (gpsimd queue)
            wo_tiles = []
            for d in range(ND):
                t = w_pool.tile([128, IL], BF16, tag="w", name=f"wo{d}")
                nc.gpsimd.dma_start(t[:], wo_v[d])
                wo_tiles.append(t)
            emit_qkproj(1, "q")
            emit_rope_pre(1, "q")
            emit_qkproj(1, "k")
            emit_rms(1, "q")
            emit_rope_pre(1, "k")
            emit_rope_final(1, "q")
            emit_rms(1, "k")
            emit_rope_final(1, "k")

            ag_outs = []

            # ---------------- Phase C0: attn0, AG0, QK2 ------------------
            load_wh(3)
            outT0 = out_pool.tile([128, S], BF16, tag="outT", name="outT0")
            emit_attn_pair(0, 0, outT0)
            emit_attn_pair(0, 1, outT0)
            emit_ag(0, outT0)
            emit_qkproj(2, "q")
            emit_rope_pre(2, "q")
            emit_qkproj(2, "k")
            emit_rms(2, "q")
            emit_rope_pre(2, "k")
            emit_rope_final(2, "q")
            emit_rms(2, "k")
            emit_rope_final(2, "k")

            # ---------------- Phase C1: attn1, QK3, attn2 ----------------
            # attn2 runs after QK3 so head-3's rope tail hides under it.
            # gathered head 0 -> SBUF (xt slots 0-3; the loads fire once
            # the QK3 matmuls release those slots)
            ag_sb = [None] * 12
            for j in range(4):
                gv = ag_outs[0].rearrange("(n p) m -> n p m", p=128)
                t = xt_pool.tile([128, S], BF16, tag="xt", name=f"agsb0_{j}")
                nc.sync.dma_start(t[:], gv[j])
                ag_sb[j] = t

            outT1 = out_pool.tile([128, S], BF16, tag="outT", name="outT1")
            ag1_outs = []
            emit_attn_pair(1, 0, outT1)
            # first half AG right after pair 0
            csl = slice(0, 2 * CHUNK)
            ag_in = dram_pool.tile([128, 2 * CHUNK], BF16, tag="agin1",
                                   bufs=2, name="agin1_0")
            nc.gpsimd.dma_start(ag_in[:], outT1[:, csl])
            ag_out = dram_pool.tile([IL, 2 * CHUNK], BF16, tag="agout1",
                                    bufs=2, name="agout1_0")
            nc.gpsimd.collective_compute(
                "AllGather", mybir.AluOpType.bypass,
                replica_groups=[[0, 1, 2, 3], [4, 5, 6, 7]],
                ins=[ag_in.opt()], outs=[ag_out.opt()])
            ag1_outs.append(ag_out)
            emit_attn_pair(1, 1, outT1)
            csl = slice(2 * CHUNK, 4 * CHUNK)
            ag_in = dram_pool.tile([128, 2 * CHUNK], BF16, tag="agin1",
                                   bufs=2, name="agin1_1")
            nc.gpsimd.dma_start(ag_in[:], outT1[:, csl])
            ag_out = dram_pool.tile([IL, 2 * CHUNK], BF16, tag="agout1",
                                    bufs=2, name="agout1_1")
            nc.gpsimd.collective_compute(
                "AllGather", mybir.AluOpType.bypass,
                replica_groups=[[0, 1, 2, 3], [4, 5, 6, 7]],
                ins=[ag_in.opt()], outs=[ag_out.opt()])
            ag1_outs.append(ag_out)

            # QK3 (k first), rope under attn2's cover
            emit_qkproj(3, "k")
            emit_rope_pre(3, "k")
            emit_qkproj(3, "q")
            emit_rms(3, "k")
            emit_rope_pre(3, "q")
            emit_rope_final(3, "k")
            emit_rms(3, "q")
            emit_rope_final(3, "q")
            # gathered head 1 -> SBUF (xt slots 4-7, freed by QK3)
            g1v = [ag1_outs[half].rearrange("(n p) m -> n p m", p=128)
                   for half in range(2)]
            for j in range(4):
                t = xt_pool.tile([128, S], BF16, tag="xt", name=f"agsb1_{j}")
                nc.sync.dma_start(t[:, 0:2 * CHUNK], g1v[0][j])
                nc.sync.dma_start(t[:, 2 * CHUNK:4 * CHUNK], g1v[1][j])
                ag_sb[4 + j] = t

            outT2 = out_pool.tile([128, S], BF16, tag="outT", name="outT2")
            ag2_outs = []
            emit_attn_pair(2, 0, outT2)
            csl = slice(0, 2 * CHUNK)
            ag_in = dram_pool.tile([128, 2 * CHUNK], BF16, tag="agin2",
                                   bufs=2, name="agin2_0")
            nc.gpsimd.dma_start(ag_in[:], outT2[:, csl])
            ag_out = dram_pool.tile([IL, 2 * CHUNK], BF16, tag="agout2",
                                    bufs=2, name="agout2_0")
            nc.gpsimd.collective_compute(
                "AllGather", mybir.AluOpType.bypass,
                replica_groups=[[0, 1, 2, 3], [4, 5, 6, 7]],
                ins=[ag_in.opt()], outs=[ag_out.opt()])
            ag2_outs.append(ag_out)
            emit_attn_pair(2, 1, outT2)
            csl = slice(2 * CHUNK, 4 * CHUNK)
            ag_in = dram_pool.tile([128, 2 * CHUNK], BF16, tag="agin2",
                                   bufs=2, name="agin2_1")
            nc.gpsimd.dma_start(ag_in[:], outT2[:, csl])
            ag_out = dram_pool.tile([IL, 2 * CHUNK], BF16, tag="agout2",
                                    bufs=2, name="agout2_1")
            nc.gpsimd.collective_compute(
                "AllGather", mybir.AluOpType.bypass,
                replica_groups=[[0, 1, 2, 3], [4, 5, 6, 7]],
                ins=[ag_in.opt()], outs=[ag_out.opt()])
            ag2_outs.append(ag_out)

            g2v = [ag2_outs[half].rearrange("(n p) m -> n p m", p=128)
                   for half in range(2)]
            for j in range(4):
                t = xt_pool.tile([128, S], BF16, tag="xt", name=f"agsb2_{j}")
                nc.sync.dma_start(t[:, 0:2 * CHUNK], g2v[0][j])
                nc.sync.dma_start(t[:, 2 * CHUNK:4 * CHUNK], g2v[1][j])
                ag_sb[8 + j] = t
            yacc = []
            for mt in range(HL):
                t = xt_pool.tile([128, S], BF16, tag="xt", name=f"yacc{mt}")
                yacc.append(t)

            # ---------------- Phase C3 -----------------------------------
            h = HL - 1
            outT3 = out_pool.tile([128, S], BF16, tag="outT", name="outT3")
            ag3_outs = []

            def emit_ag3(half):
                csl = slice(half * 2 * CHUNK, (half + 1) * 2 * CHUNK)
                ag_in = dram_pool.tile([128, 2 * CHUNK], BF16, tag="agin3",
                                       bufs=2, name=f"agin3_{half}")
                nc.gpsimd.dma_start(ag_in[:], outT3[:, csl])
                ag_out = dram_pool.tile([IL, 2 * CHUNK], BF16, tag="agout3",
                                        bufs=2, name=f"agout3_{half}")
                nc.gpsimd.collective_compute(
                    "AllGather",
                    mybir.AluOpType.bypass,
                    replica_groups=[[0, 1, 2, 3], [4, 5, 6, 7]],
                    ins=[ag_in.opt()],
                    outs=[ag_out.opt()],
                )
                ag3_outs.append(ag_out)

            emit_attn_pair(h, 0, outT3)
            emit_ag3(0)
            emit_attn_pair(h, 1, outT3)
            emit_ag3(1)

            # partial out-proj over gathered heads 0-2 -> yacc (bf16)
            for mt in range(HL):
                msl = slice(mt * 128, (mt + 1) * 128)
                st = ps_score(f"yp{mt}_hi")
                yps = [ps_qk(f"yp{mt}_0"), ps_pvden(f"yp{mt}_1"),
                       st[:, 0:CHUNK], st[:, CHUNK:2 * CHUNK]]
                for gj in range(12):
                    lhsT = wo_tiles[gj][:, msl]
                    for c in range(NC):
                        csl = slice(c * CHUNK, (c + 1) * CHUNK)
                        mm(yps[c][:], lhsT, ag_sb[gj][:, csl],
                           start=(gj == 0), stop=(gj == 11), reuse=(c > 0))
                for c in range(NC):
                    csl = slice(c * CHUNK, (c + 1) * CHUNK)
                    nc.vector.tensor_copy(yacc[mt][:, csl], yps[c][:])

            # ---------------- Tail: head-3 out-proj + store --------------
            ag3_sb = []
            for half in range(2):
                gv = ag3_outs[half].rearrange("(n p) m -> n p m", p=128)
                for j in range(4):
                    t = xt_pool.tile([128, 2 * CHUNK], BF16, tag="xt",
                                     name=f"ag3sb{half}_{j}")
                    nc.sync.dma_start(t[:], gv[j])
                    ag3_sb.append(t)

            for mt in range(HL):
                msl = slice(mt * 128, (mt + 1) * 128)
                st = ps_score(f"z{mt}_hi")
                yps = [ps_qk(f"z{mt}_0"), ps_pvden(f"z{mt}_1"),
                       st[:, 0:CHUNK], st[:, CHUNK:2 * CHUNK]]
                for j in range(4):
                    lhsT = wo_tiles[12 + j][:, msl]
                    for c in range(NC):
                        src = ag3_sb[(c // 2) * 4 + j][:, (c % 2) * CHUNK:
                                                       (c % 2 + 1) * CHUNK]
                        mm(yps[c][:], lhsT, src,
                           start=(j == 0), stop=(j == 3), reuse=(c > 0))
                for c in range(NC):
                    csl = slice(c * CHUNK, (c + 1) * CHUNK)
                    ysb = small_pool.tile([128, CHUNK], BF16, tag="ysb",
                                          bufs=4)
                    nc.vector.tensor_tensor(ysb[:], yps[c][:],
                                            yacc[mt][:, csl], op=ADD)
                    nc.sync.dma_start(yT.ap()[msl, csl], ysb[:])

    _dedupe_ldweights(nc)
    nc.finalize()
    return nc


# psum-tile name prefixes whose accumulation chains are safe to merge:
# nothing waits on their interior matmuls' semaphore values except
# long-delay buffer-reuse edges (wo prefetch, ag3 staging loads).
_MERGE_PREFIXES = ("ac_", "vps", "aq", "ak", "yp", "z")


def _merge_chain_updates(nc):
    """Move interior accumulation-chain matmul semaphore increments onto the
    chain's stop matmul (one sem-add-imm instead of N serialized sem-incs).
    The engine pays ~26ns per semaphore write; chains are 12-16 matmuls."""
    for f in nc.m.functions:
        for b in f.blocks:
            cur = {}
            chains = []
            for i in b.instructions:
                if not isinstance(i, mybir.InstMatmult):
                    continue
                out = i.outs[0]
                ref = getattr(out, "memref", None)
                if ref is None or not ref.startswith(_MERGE_PREFIXES):
                    continue
                start = bool(i.start_tensor_calc)
                stop = bool(i.stop_tensor_calc)
                if start and stop:
                    continue
                if start:
                    cur[ref] = [i]
                elif ref in cur:
                    cur[ref].append(i)
                    if stop:
                        chains.append(cur.pop(ref))
            for mem in chains:
                stop_i = mem[-1]
                ssi = stop_i.sync_info
                if ssi is None or len(ssi.on_update) != 1:
                    continue
                su = ssi.on_update[0]
                if su.sync_type != "semaphore" or su.update_mode not in (
                        "sem-inc", "sem-add-imm"):
                    continue
                moved = 0
                for i in mem[:-1]:
                    si = i.sync_info
                    if si is None or len(si.on_update) != 1:
                        continue
                    u = si.on_update[0]
                    if (u.sync_type == "semaphore" and u.id == su.id
                            and u.update_mode == "sem-inc"
                            and u.update_value == 1):
                        i.sync_info = mybir.SyncInfo(
                            on_wait=list(si.on_wait), on_update=[])
                        moved += 1
                if moved:
                    ssi.on_update = [mybir.SyncUpdate(
                        sync_type=su.sync_type, id=su.id,
                        ant_name=su.ant_name, update_mode="sem-add-imm",
                        update_value=su.update_value + moved,
                        update_reg=su.update_reg)]
                    stop_i.sync_info = ssi


def _ldw_sig(i):
    ap = i.ins[0]
    try:
        return (ap.memref, ap.offset, str(ap.ap), str(ap.dtype),
                str(i.perf_mode), str(i.tile_position), str(i.tile_size),
                bool(i.is_transpose))
    except Exception:
        return None


def _dedupe_ldweights(nc):
    """Drop InstLdweights that reload the exact weights already resident in
    the PE array (emitted per-matmul by the framework even when consecutive
    matmuls share the stationary operand). Only sync-free repeats are
    removed, so all semaphore waits/updates are preserved."""
    for f in nc.m.functions:
        for b in f.blocks:
            cur = None
            keep = []
            for i in b.instructions:
                if isinstance(i, mybir.InstLdweights):
                    sig = _ldw_sig(i)
                    si = i.sync_info
                    clean = si is None or (len(si.on_wait) == 0
                                           and len(si.on_update) == 0)
                    if sig is not None and sig == cur and clean:
                        continue
                    cur = sig
                elif isinstance(i, mybir.InstMatmult):
                    if i.is_transpose:
                        cur = None
                keep.append(i)
            if len(keep) != len(b.instructions):
                b.instructions = keep


# inner-dim permutation for per-head AllGather order:
# block (g, j) of gathered = rank j's local head g = global inner
# [(4*j + g)*128 : (4*j + g + 1)*128]
_WO_PERM = np.concatenate(
    [np.arange(128) + (4 * j + g) * 128 for g in range(4) for j in range(4)])


def _host_prep(x, rope_emb, w_q, w_k, w_v, w_o, q_norm_w, k_norm_w):
    """Build the 8 per-core input maps."""
    f = rope_emb[:, 0].astype(np.float32)  # [S, 64, 2, 2]

    def coeffs(w):
        ca = np.empty((HD, S), np.float32)
        cb = np.empty((HD, S), np.float32)
        ca[0:64] = f[:, :, 0, 0].T * w[0:64, None]
        ca[64:128] = f[:, :, 1, 1].T * w[64:128, None]
        cb[0:64] = f[:, :, 0, 1].T * w[64:128, None]
        cb[64:128] = f[:, :, 1, 0].T * w[0:64, None]
        return ca.astype(BF), cb.astype(BF)

    caq, cbq = coeffs(q_norm_w.astype(np.float32))
    cak, cbk = coeffs(k_norm_w.astype(np.float32))

    def pack_wh(w_slice):
        # w_slice: [IL, DM], head-major rows. Output [128, HL*ND*128]
        # with value[p, h*2048 + d*128 + c] = w_slice[h*128+c, d*128+p].
        wt = w_slice.T.astype(np.float32)          # [DM, IL]
        out = np.empty((128, HL * ND * 128), np.float32)
        for hh in range(HL):
            blk = wt[:, hh * 128:(hh + 1) * 128]   # [DM, 128]
            blk = blk.reshape(ND, 128, 128).transpose(1, 0, 2)
            out[:, hh * ND * 128:(hh + 1) * ND * 128] = blk.reshape(128, -1)
        return out.astype(BF)

    in_maps = []
    for c in range(8):
        b, hg = c // 4, c % 4
        sl = slice(IL * hg, IL * (hg + 1))
        in_maps.append({
            "xT": np.ascontiguousarray(x[b].T).astype(BF),
            "whq": pack_wh(w_q[sl]),
            "whk": pack_wh(w_k[sl]),
            "wvT": np.ascontiguousarray(w_v[sl].T).astype(BF),
            "woT": np.ascontiguousarray(w_o[sl][:, _WO_PERM].T).astype(BF),
            "caq": caq, "cbq": cbq, "cak": cak, "cbk": cbk,
        })
    return in_maps


def kernel(x, rope_emb, w_q, w_k, w_v, w_o, q_norm_w, k_norm_w, trace=False):
    x = np.asarray(x, dtype=np.float32)
    rope_emb = np.asarray(rope_emb, dtype=np.float32)
    w_q = np.asarray(w_q, dtype=np.float32)
    w_k = np.asarray(w_k, dtype=np.float32)
    w_v = np.asarray(w_v, dtype=np.float32)
    w_o = np.asarray(w_o, dtype=np.float32)
    q_norm_w = np.asarray(q_norm_w, dtype=np.float32)
    k_norm_w = np.asarray(k_norm_w, dtype=np.float32)
    if "nc" not in _CACHED:
        _CACHED["nc"] = _build_nc()
    nc = _CACHED["nc"]
    in_maps = _host_prep(x, rope_emb, w_q, w_k, w_v, w_o, q_norm_w, k_norm_w)
    res = run_bass_kernel_spmd(nc, in_maps, core_ids=list(range(8)),
                               trace=trace)
    _CACHED["last_result"] = res
    y = np.empty((B, S, DM), np.float32)
    for c in range(8):
        b, hg = c // 4, c % 4
        y[b, :, IL * hg:IL * (hg + 1)] = res.results[c]["yT"].T.astype(np.float32)
    return y


# revision 42
# speedup vs baseline: 1.0454x; 1.0245x over previous
"""CosmosAttention distributed Bass kernel for 8 TRN2 NeuronCores.

Sharding: 8 cores = 2 batches x 4 head-groups (tensor-parallel over heads,
data-parallel over batch). Each core computes QKV + attention for its 4 heads
of its batch, AllGathers attention outputs within its 4-core batch group, and
computes a 512-wide slice of the output projection.

Pipeline (per core), software-pipelined two phases deep so attention never
waits on the rope tail of the same head:
  A:  QK-proj head 0, weight-stationary over d, paced by the xT DMA stream
  B:  V-proj, rope head 0, QK-proj + rope head 1
  C0: attn0, AG0, QK-proj + rope head 2
  C1: attn1, AG1, attn2, AG2, QK-proj + rope head 3
  C3: attn3 (2 chunk-pairs, AllGathered per pair), partial out-proj over
      gathered heads 0-2, final out-proj of head 3, y store

Stationary-weight reuse: matmuls that repeat the previous lhsT are emitted
with InstMatmult.ldweights=False so the PE array skips the weight reload.

PSUM budget (8 banks): tags qk 2x[128,512] + score 2x[128,1024] +
pvden 2x[128,512].
"""

import numpy as np
import ml_dtypes

import concourse.bass as bass
import concourse.mybir as mybir
import concourse.tile as tile
from concourse import bacc
from concourse.bass_utils import run_bass_kernel_spmd

F32 = mybir.dt.float32
BF16 = mybir.dt.bfloat16
BF = ml_dtypes.bfloat16

B, S, DM, H, HD = 2, 2048, 2048, 16, 128
HL = 4           # heads per core
IL = HL * HD     # inner slice per core = 512
ND = DM // 128   # 16 d-tiles
NC = 4           # token chunks of 512
NTB = S // 128   # 16 token blocks
CHUNK = 512
EPS = 1e-6
INV_SQRT_HD = 1.0 / float(np.sqrt(HD))
MULT = mybir.AluOpType.mult
ADD = mybir.AluOpType.add
REUSE_LDW = True

_CACHED = {}


def _build_nc():
    nc = bacc.Bacc("TRN2", target_bir_lowering=False, debug=False, num_devices=8)

    xT = nc.dram_tensor("xT", [DM, S], BF16, kind="ExternalInput")
    wvT = nc.dram_tensor("wvT", [DM, IL], BF16, kind="ExternalInput")
    woT = nc.dram_tensor("woT", [DM, IL], BF16, kind="ExternalInput")
    whq = nc.dram_tensor("whq", [128, HL * ND * 128], BF16, kind="ExternalInput")
    whk = nc.dram_tensor("whk", [128, HL * ND * 128], BF16, kind="ExternalInput")
    caq = nc.dram_tensor("caq", [HD, S], BF16, kind="ExternalInput")
    cbq = nc.dram_tensor("cbq", [HD, S], BF16, kind="ExternalInput")
    cak = nc.dram_tensor("cak", [HD, S], BF16, kind="ExternalInput")
    cbk = nc.dram_tensor("cbk", [HD, S], BF16, kind="ExternalInput")
    yT = nc.dram_tensor("yT", [IL, S], BF16, kind="ExternalOutput")

    xT_v = xT.ap().rearrange("(n p) m -> n p m", p=128)
    wv_v = wvT.ap().rearrange("(n p) m -> n p m", p=128)
    wo_v = woT.ap().rearrange("(n p) m -> n p m", p=128)

    def mm(out, lhsT, rhs, start, stop, reuse=False):
        bi = nc.tensor.matmul(out, lhsT, rhs, start=start, stop=stop)
        if reuse and REUSE_LDW:
            bi.ins.ldweights = False
        return bi

    import contextlib
    with tile.TileContext(nc) as tc:
        with contextlib.ExitStack() as _stk:
            def _pool(name, bufs, space="SBUF"):
                return _stk.enter_context(
                    tc.tile_pool(name=name, bufs=bufs, space=space))
            const_pool = _pool("const", 1)
            xt_pool = _pool("xt", ND)
            w_pool = _pool("w", ND)
            wh_pool = _pool("wh", 2)
            cab_pool = _pool("cab", 4)
            qk_pool = _pool("qk", 4)
            v_pool = _pool("v", NTB)
            raw_pool = _pool("raw", 2)
            pre_pool = _pool("pre", 3)
            tmp_pool = _pool("tmp", 2)
            rstd_pool = _pool("rstd", 1)
            sstd_pool = _pool("sstd", 1)
            e_pool = _pool("e", 3)
            ep_pool = _pool("ep", 3)
            small_pool = _pool("small", 2)
            out_pool = _pool("outT", 2)
            ps_pool = _pool("ps", 2, space="PSUM")
            dram_pool = _pool("dram", 1, space="DRAM")

            ones128 = const_pool.tile([128, 128], BF16)
            nc.vector.memset(ones128[:], 1.0)
            eps_tile = const_pool.tile([128, 1], F32)
            nc.vector.memset(eps_tile[:], EPS)

            def ps_qk(name):
                return ps_pool.tile([128, CHUNK], F32, tag="qk", bufs=2,
                                    name=name)

            def ps_pvden(name):
                return ps_pool.tile([128, CHUNK], F32, tag="pvden", bufs=2,
                                    name=name)

            def ps_score(name):
                return ps_pool.tile([128, 2 * CHUNK], F32, tag="score",
                                    bufs=2, name=name)

            # ---------------- input DMAs (sync queue, ordered) -----------
            wh_tiles = {}

            def load_wh(h):
                for part, src in (("q", whq), ("k", whk)):
                    t = wh_pool.tile([128, ND * 128], BF16, tag="wh",
                                     name=f"wh_{part}{h}")
                    nc.sync.dma_start(
                        t[:], src.ap()[:, h * ND * 128:(h + 1) * ND * 128])
                    wh_tiles[(part, h)] = t

            # whq0 first, then xt[0], then whk0 so the first q-matmul can
            # start as early as possible
            t = wh_pool.tile([128, ND * 128], BF16, tag="wh", name="wh_q0")
            nc.sync.dma_start(t[:, 0:4 * 128], whq.ap()[:, 0:4 * 128])
            nc.sync.dma_start(t[:, 4 * 128:ND * 128],
                              whq.ap()[:, 4 * 128:ND * 128])
            wh_tiles[("q", 0)] = t
            xt = []
            t = xt_pool.tile([128, S], BF16, tag="xt", name="xt0")
            nc.sync.dma_start(t[:], xT_v[0])
            xt.append(t)
            t = wh_pool.tile([128, ND * 128], BF16, tag="wh", name="wh_k0")
            nc.sync.dma_start(t[:], whk.ap()[:, 0:ND * 128])
            wh_tiles[("k", 0)] = t
            for d in range(1, ND):
                t = xt_pool.tile([128, S], BF16, tag="xt", name=f"xt{d}")
                nc.sync.dma_start(t[:], xT_v[d])
                xt.append(t)
            wv_tiles = []
            for d in range(ND):
                t = w_pool.tile([128, IL], BF16, tag="w", name=f"wv{d}")
                nc.sync.dma_start(t[:], wv_v[d])
                wv_tiles.append(t)
            ca_q = cab_pool.tile([HD, S], BF16, tag="cab")
            cb_q = cab_pool.tile([HD, S], BF16, tag="cab")
            ca_k = cab_pool.tile([HD, S], BF16, tag="cab")
            cb_k = cab_pool.tile([HD, S], BF16, tag="cab")
            nc.sync.dma_start(ca_q[:], caq.ap())
            nc.sync.dma_start(cb_q[:], cbq.ap())
            nc.sync.dma_start(ca_k[:], cak.ap())
            nc.sync.dma_start(cb_k[:], cbk.ap())
            cab = {"q": (ca_q, cb_q), "k": (ca_k, cb_k)}
            load_wh(1)

            qk_tiles = {}
            raw_tiles = {}
            v_sb = [None] * NTB

            # ---------------- helpers ------------------------------------
            def emit_qkproj(h, part):
                """Projection for one head/part: two chunk-pair passes,
                lhsT reused across the pair."""
                wh = wh_tiles[(part, h)]
                raw = raw_pool.tile([128, S], BF16, tag="raw",
                                    name=f"raw_{part}{h}")
                for cp, mk in ((0, ps_qk), (2, ps_pvden)):
                    acc0 = mk(f"ac_{part}{h}_{cp}")
                    acc1 = mk(f"ac_{part}{h}_{cp + 1}")
                    for d in range(ND):
                        lhsT = wh[:, d * 128:(d + 1) * 128]
                        mm(acc0[:], lhsT,
                           xt[d][:, cp * CHUNK:(cp + 1) * CHUNK],
                           start=(d == 0), stop=(d == ND - 1))
                        mm(acc1[:], lhsT,
                           xt[d][:, (cp + 1) * CHUNK:(cp + 2) * CHUNK],
                           start=(d == 0), stop=(d == ND - 1), reuse=True)
                    with tc.high_priority(offset=150):
                        nc.vector.tensor_copy(
                            raw[:, cp * CHUNK:(cp + 1) * CHUNK], acc0[:])
                        nc.vector.tensor_copy(
                            raw[:, (cp + 1) * CHUNK:(cp + 2) * CHUNK],
                            acc1[:])
                raw_tiles[(part, h)] = raw

            rope_pre = {}
            rstd_tiles = {}

            def emit_rope_pre(h, part):
                """Early rope work (DVE square + swap DMA) emitted right
                after the projection so it overlaps the next unit's
                matmuls."""
                raw = raw_tiles[(part, h)]
                sq = pre_pool.tile([128, S], BF16, tag="pre",
                                   name=f"sq_{part}{h}")
                with tc.high_priority(offset=150):
                    nc.vector.tensor_tensor(sq[:], raw[:], raw[:], op=MULT)
                swap = pre_pool.tile([128, S], BF16, tag="pre",
                                     name=f"swap_{part}{h}")
                nc.sync.dma_start(swap[0:64, :], raw[64:128, :])
                nc.sync.dma_start(swap[64:128, :], raw[0:64, :])
                rope_pre[(part, h)] = (sq, swap)

            def emit_rms(h, part):
                """rms scale: ss matmuls + sqrt (scalar) + reciprocal."""
                sq, _ = rope_pre[(part, h)]
                rstd = rstd_pool.tile([128, S], F32, tag="rstd",
                                      name=f"rstd_{part}{h}")
                for half in range(2):
                    st = ps_score(f"ss_{part}{h}_{half}")
                    for i in range(2):
                        c = 2 * half + i
                        mm(st[:, i * CHUNK:(i + 1) * CHUNK], ones128[:],
                           sq[:, c * CHUNK:(c + 1) * CHUNK],
                           start=True, stop=True, reuse=(c > 0))
                    sstd = sstd_pool.tile([128, 2 * CHUNK], F32, tag="sstd",
                                          name=f"sstd_{part}{h}_{half}")
                    nc.scalar.activation(
                        sstd[:], st[:], mybir.ActivationFunctionType.Sqrt,
                        bias=eps_tile[:], scale=1.0 / HD)
                    with tc.high_priority(offset=150):
                        nc.vector.reciprocal_approx_fast(
                            rstd[:, half * 2 * CHUNK:(half + 1) * 2 * CHUNK],
                            sstd[:])
                rstd_tiles[(part, h)] = rstd

            def emit_rope_final(h, part):
                """rope rotation + rms scale, halves chunked so the first
                1024 columns of the destination land early."""
                raw = raw_tiles[(part, h)]
                ca_t, cb_t = cab[part]
                _, swap = rope_pre[(part, h)]
                rstd = rstd_tiles[(part, h)]
                r1 = tmp_pool.tile([128, S], BF16, tag="tmp",
                                   name=f"r1_{part}{h}")
                r2 = tmp_pool.tile([128, S], BF16, tag="tmp",
                                   name=f"r2_{part}{h}")
                dst = qk_pool.tile([128, S], BF16, tag="qk",
                                   name=f"qk_{part}{h}")
                with tc.high_priority(offset=150):
                    nc.vector.tensor_tensor(r1[:], raw[:], ca_t[:], op=MULT)
                    nc.vector.tensor_tensor(r2[:], swap[:], cb_t[:], op=MULT)
                    for half in range(2):
                        hs = slice(half * 2 * CHUNK, (half + 1) * 2 * CHUNK)
                        nc.vector.tensor_tensor(r1[:, hs], r1[:, hs],
                                                r2[:, hs], op=ADD)
                        nc.vector.tensor_tensor(dst[:, hs], r1[:, hs],
                                                rstd[:, hs], op=MULT)
                qk_tiles[(part, h)] = dst

            def emit_attn_pair(h, pair, outT_h):
                """Attention for head h, query chunks (2*pair, 2*pair+1).
                kh[tb] / v[tb] stationary reused across the chunk pair;
                software-pipelined one tb so PV never waits on exp."""
                hsl = slice(h * 128, (h + 1) * 128)
                sa = slice(2 * pair * CHUNK, (2 * pair + 1) * CHUNK)
                sb = slice((2 * pair + 1) * CHUNK, (2 * pair + 2) * CHUNK)
                qh = qk_tiles[("q", h)]
                kh = qk_tiles[("k", h)]
                pv_a = ps_pvden(f"pva{h}_{pair}")
                pv_b = ps_pvden(f"pvb{h}_{pair}")
                den_a = ps_qk(f"dena{h}_{pair}")
                den_b = ps_qk(f"denb{h}_{pair}")
                prev = None
                e_hold = None
                ep_hold = None
                for tb in range(NTB):
                    st = ps_score(f"s{h}_{pair}_{tb}")
                    khs = kh[:, tb * 128:(tb + 1) * 128]
                    mm(st[:, 0:CHUNK], khs, qh[:, sa], start=True, stop=True)
                    mm(st[:, CHUNK:2 * CHUNK], khs, qh[:, sb],
                       start=True, stop=True, reuse=True)
                    e = e_pool.tile([128, 2 * CHUNK], BF16, tag="e", bufs=3)
                    nc.scalar.activation(e[:], st[:],
                                         mybir.ActivationFunctionType.Exp,
                                         bias=0.0, scale=INV_SQRT_HD)
                    if prev is not None:
                        ptb, pe = prev
                        vt = v_sb[ptb][:, hsl]
                        mm(pv_a[:], vt, pe[:, 0:CHUNK],
                           start=(ptb == 0), stop=False)
                        mm(pv_b[:], vt, pe[:, CHUNK:2 * CHUNK],
                           start=(ptb == 0), stop=False, reuse=True)
                    prev = (tb, e)
                    if tb % 2 == 0:
                        e_hold = e
                    else:
                        ep = ep_pool.tile([128, 2 * CHUNK], BF16, tag="ep",
                                          bufs=3)
                        nc.vector.tensor_tensor(ep[:], e_hold[:], e[:],
                                                op=ADD)
                        mm(den_a[:], ones128[:], ep[:, 0:CHUNK],
                           start=(tb == 1), stop=(tb == NTB - 1))
                        mm(den_b[:], ones128[:], ep[:, CHUNK:2 * CHUNK],
                           start=(tb == 1), stop=(tb == NTB - 1),
                           reuse=True)
                ptb, pe = prev
                vt = v_sb[ptb][:, hsl]
                mm(pv_a[:], vt, pe[:, 0:CHUNK], start=False, stop=True)
                mm(pv_b[:], vt, pe[:, CHUNK:2 * CHUNK], start=False,
                   stop=True, reuse=True)
                for pv, den, ssl in ((pv_a, den_a, sa), (pv_b, den_b, sb)):
                    rden = small_pool.tile([128, CHUNK], F32, tag="rden")
                    nc.vector.reciprocal_approx_fast(rden[:], den[:])
                    nc.vector.tensor_tensor(outT_h[:, ssl], pv[:], rden[:],
                                            op=MULT)

            def emit_ag(h, outT_h):
                ag_in = dram_pool.tile([128, S], BF16, tag="agin",
                                       bufs=HL - 1, name=f"agin{h}")
                nc.gpsimd.dma_start(ag_in[:], outT_h[:])
                ag_out = dram_pool.tile([IL, S], BF16, tag="agout",
                                        bufs=HL - 1, name=f"agout{h}")
                nc.gpsimd.collective_compute(
                    "AllGather",
                    mybir.AluOpType.bypass,
                    replica_groups=[[0, 1, 2, 3], [4, 5, 6, 7]],
                    ins=[ag_in.opt()],
                    outs=[ag_out.opt()],
                )
                ag_outs.append(ag_out)

            # ---------------- Phase A: QK proj head 0 (paced) -----------
            a_accs = {}
            ss0 = ps_score("aq01")
            ss1 = ps_score("aq23")
            a_accs[("q", 0)] = ss0[:, 0:CHUNK]
            a_accs[("q", 1)] = ss0[:, CHUNK:2 * CHUNK]
            a_accs[("q", 2)] = ss1[:, 0:CHUNK]
            a_accs[("q", 3)] = ss1[:, CHUNK:2 * CHUNK]
            a_accs[("k", 0)] = ps_qk("ak0")[:]
            a_accs[("k", 1)] = ps_qk("ak1")[:]
            a_accs[("k", 2)] = ps_pvden("ak2")[:]
            a_accs[("k", 3)] = ps_pvden("ak3")[:]
            for d in range(ND):
                for part in ("q", "k"):
                    lhsT = wh_tiles[(part, 0)][:, d * 128:(d + 1) * 128]
                    for c in range(NC):
                        csl = slice(c * CHUNK, (c + 1) * CHUNK)
                        mm(a_accs[(part, c)], lhsT, xt[d][:, csl],
                           start=(d == 0), stop=(d == ND - 1), reuse=(c > 0))
            for part in ("k", "q"):
                raw = raw_pool.tile([128, S], BF16, tag="raw",
                                    name=f"raw_{part}0")
                for c in range(NC):
                    csl = slice(c * CHUNK, (c + 1) * CHUNK)
                    nc.vector.tensor_copy(raw[:, csl], a_accs[(part, c)])
                raw_tiles[(part, 0)] = raw
                emit_rope_pre(0, part)

            # ---------------- Phase B: V proj, rope0, QK1 ----------------
            load_wh(2)
            for tb in range(NTB):
                tsl = slice(tb * 128, (tb + 1) * 128)
                ps = ps_qk(f"vps{tb}") if tb % 2 == 0 else ps_pvden(f"vps{tb}")
                for d in range(ND):
                    nc.tensor.matmul(ps[:], xt[d][:, tsl], wv_tiles[d][:],
                                     start=(d == 0), stop=(d == ND - 1))
                vt = v_pool.tile([128, IL], BF16, tag="v", name=f"v{tb}")
                nc.vector.tensor_copy(vt[:], ps[:])
                v_sb[tb] = vt
                if tb == 5:
                    emit_rms(0, "q")
                    emit_rope_final(0, "q")
                if tb == 9:
                    emit_rms(0, "k")
                    emit_rope_final(0, "k")
            # wo prefetch through freed wv slots (gpsimd queue)
            wo_tiles = []
            for d in range(ND):
                t = w_pool.tile([128, IL], BF16, tag="w", name=f"wo{d}")
                nc.gpsimd.dma_start(t[:], wo_v[d])
                wo_tiles.append(t)
            emit_qkproj(1, "q")
            emit_rope_pre(1, "q")
            emit_qkproj(1, "k")
            emit_rope_pre(1, "k")

            ag_outs = []

            # ---------------- Phase C0: attn0, AG0, QK2 ------------------
            load_wh(3)
            outT0 = out_pool.tile([128, S], BF16, tag="outT", name="outT0")
            emit_attn_pair(0, 0, outT0)
            emit_rms(1, "q")
            emit_rope_final(1, "q")
            emit_attn_pair(0, 1, outT0)
            emit_rms(1, "k")
            emit_rope_final(1, "k")
            emit_ag(0, outT0)
            emit_qkproj(2, "q")
            emit_rope_pre(2, "q")
            emit_qkproj(2, "k")
            emit_rope_pre(2, "k")

            # ---------------- Phase C1: attn1, QK3, attn2 ----------------
            # attn2 runs after QK3 so head-3's rope tail hides under it.
            # gathered head 0 -> SBUF (xt slots 0-3; the loads fire once
            # the QK3 matmuls release those slots)
            ag_sb = [None] * 12
            for j in range(4):
                gv = ag_outs[0].rearrange("(n p) m -> n p m", p=128)
                t = xt_pool.tile([128, S], BF16, tag="xt", name=f"agsb0_{j}")
                nc.sync.dma_start(t[:], gv[j])
                ag_sb[j] = t

            outT1 = out_pool.tile([128, S], BF16, tag="outT", name="outT1")
            ag1_outs = []
            emit_attn_pair(1, 0, outT1)
            emit_rms(2, "q")
            emit_rope_final(2, "q")
            # first half AG right after pair 0
            csl = slice(0, 2 * CHUNK)
            ag_in = dram_pool.tile([128, 2 * CHUNK], BF16, tag="agin1",
                                   bufs=2, name="agin1_0")
            nc.gpsimd.dma_start(ag_in[:], outT1[:, csl])
            ag_out = dram_pool.tile([IL, 2 * CHUNK], BF16, tag="agout1",
                                    bufs=2, name="agout1_0")
            nc.gpsimd.collective_compute(
                "AllGather", mybir.AluOpType.bypass,
                replica_groups=[[0, 1, 2, 3], [4, 5, 6, 7]],
                ins=[ag_in.opt()], outs=[ag_out.opt()])
            ag1_outs.append(ag_out)
            emit_attn_pair(1, 1, outT1)
            emit_rms(2, "k")
            emit_rope_final(2, "k")
            csl = slice(2 * CHUNK, 4 * CHUNK)
            ag_in = dram_pool.tile([128, 2 * CHUNK], BF16, tag="agin1",
                                   bufs=2, name="agin1_1")
            nc.gpsimd.dma_start(ag_in[:], outT1[:, csl])
            ag_out = dram_pool.tile([IL, 2 * CHUNK], BF16, tag="agout1",
                                    bufs=2, name="agout1_1")
            nc.gpsimd.collective_compute(
                "AllGather", mybir.AluOpType.bypass,
                replica_groups=[[0, 1, 2, 3], [4, 5, 6, 7]],
                ins=[ag_in.opt()], outs=[ag_out.opt()])
            ag1_outs.append(ag_out)

            # QK3 (k first); its rope is emitted inside attn2 below
            emit_qkproj(3, "k")
            emit_rope_pre(3, "k")
            emit_qkproj(3, "q")
            emit_rope_pre(3, "q")
            # gathered head 1 -> SBUF (xt slots 4-7, freed by QK3)
            g1v = [ag1_outs[half].rearrange("(n p) m -> n p m", p=128)
                   for half in range(2)]
            for j in range(4):
                t = xt_pool.tile([128, S], BF16, tag="xt", name=f"agsb1_{j}")
                nc.sync.dma_start(t[:, 0:2 * CHUNK], g1v[0][j])
                nc.sync.dma_start(t[:, 2 * CHUNK:4 * CHUNK], g1v[1][j])
                ag_sb[4 + j] = t

            outT2 = out_pool.tile([128, S], BF16, tag="outT", name="outT2")
            ag2_outs = []
            emit_attn_pair(2, 0, outT2)
            emit_rms(3, "k")
            emit_rope_final(3, "k")
            csl = slice(0, 2 * CHUNK)
            ag_in = dram_pool.tile([128, 2 * CHUNK], BF16, tag="agin2",
                                   bufs=2, name="agin2_0")
            nc.gpsimd.dma_start(ag_in[:], outT2[:, csl])
            ag_out = dram_pool.tile([IL, 2 * CHUNK], BF16, tag="agout2",
                                    bufs=2, name="agout2_0")
            nc.gpsimd.collective_compute(
                "AllGather", mybir.AluOpType.bypass,
                replica_groups=[[0, 1, 2, 3], [4, 5, 6, 7]],
                ins=[ag_in.opt()], outs=[ag_out.opt()])
            ag2_outs.append(ag_out)
            emit_attn_pair(2, 1, outT2)
            emit_rms(3, "q")
            emit_rope_final(3, "q")
            csl = slice(2 * CHUNK, 4 * CHUNK)
            ag_in = dram_pool.tile([128, 2 * CHUNK], BF16, tag="agin2",
                                   bufs=2, name="agin2_1")
            nc.gpsimd.dma_start(ag_in[:], outT2[:, csl])
            ag_out = dram_pool.tile([IL, 2 * CHUNK], BF16, tag="agout2",
                                    bufs=2, name="agout2_1")
            nc.gpsimd.collective_compute(
                "AllGather", mybir.AluOpType.bypass,
                replica_groups=[[0, 1, 2, 3], [4, 5, 6, 7]],
                ins=[ag_in.opt()], outs=[ag_out.opt()])
            ag2_outs.append(ag_out)

            g2v = [ag2_outs[half].rearrange("(n p) m -> n p m", p=128)
                   for half in range(2)]
            for j in range(4):
                t = xt_pool.tile([128, S], BF16, tag="xt", name=f"agsb2_{j}")
                nc.sync.dma_start(t[:, 0:2 * CHUNK], g2v[0][j])
                nc.sync.dma_start(t[:, 2 * CHUNK:4 * CHUNK], g2v[1][j])
                ag_sb[8 + j] = t
            yacc = []
            for mt in range(HL):
                t = xt_pool.tile([128, S], BF16, tag="xt", name=f"yacc{mt}")
                yacc.append(t)

            # ---------------- Phase C3 -----------------------------------
            h = HL - 1
            outT3 = out_pool.tile([128, S], BF16, tag="outT", name="outT3")
            ag3_outs = []

            def emit_ag3(half):
                csl = slice(half * 2 * CHUNK, (half + 1) * 2 * CHUNK)
                ag_in = dram_pool.tile([128, 2 * CHUNK], BF16, tag="agin3",
                                       bufs=2, name=f"agin3_{half}")
                nc.gpsimd.dma_start(ag_in[:], outT3[:, csl])
                ag_out = dram_pool.tile([IL, 2 * CHUNK], BF16, tag="agout3",
                                        bufs=2, name=f"agout3_{half}")
                nc.gpsimd.collective_compute(
                    "AllGather",
                    mybir.AluOpType.bypass,
                    replica_groups=[[0, 1, 2, 3], [4, 5, 6, 7]],
                    ins=[ag_in.opt()],
                    outs=[ag_out.opt()],
                )
                ag3_outs.append(ag_out)

            emit_attn_pair(h, 0, outT3)
            emit_ag3(0)
            emit_attn_pair(h, 1, outT3)
            emit_ag3(1)

            # partial out-proj over gathered heads 0-2 -> yacc (bf16)
            for mt in range(HL):
                msl = slice(mt * 128, (mt + 1) * 128)
                st = ps_score(f"yp{mt}_hi")
                yps = [ps_qk(f"yp{mt}_0"), ps_pvden(f"yp{mt}_1"),
                       st[:, 0:CHUNK], st[:, CHUNK:2 * CHUNK]]
                for gj in range(12):
                    lhsT = wo_tiles[gj][:, msl]
                    for c in range(NC):
                        csl = slice(c * CHUNK, (c + 1) * CHUNK)
                        mm(yps[c][:], lhsT, ag_sb[gj][:, csl],
                           start=(gj == 0), stop=(gj == 11), reuse=(c > 0))
                for c in range(NC):
                    csl = slice(c * CHUNK, (c + 1) * CHUNK)
                    nc.vector.tensor_copy(yacc[mt][:, csl], yps[c][:])

            # ---------------- Tail: head-3 out-proj + store --------------
            ag3_sb = []
            for half in range(2):
                gv = ag3_outs[half].rearrange("(n p) m -> n p m", p=128)
                for j in range(4):
                    t = xt_pool.tile([128, 2 * CHUNK], BF16, tag="xt",
                                     name=f"ag3sb{half}_{j}")
                    nc.sync.dma_start(t[:], gv[j])
                    ag3_sb.append(t)

            for mt in range(HL):
                msl = slice(mt * 128, (mt + 1) * 128)
                st = ps_score(f"z{mt}_hi")
                yps = [ps_qk(f"z{mt}_0"), ps_pvden(f"z{mt}_1"),
                       st[:, 0:CHUNK], st[:, CHUNK:2 * CHUNK]]
                for j in range(4):
                    lhsT = wo_tiles[12 + j][:, msl]
                    for c in range(NC):
                        src = ag3_sb[(c // 2) * 4 + j][:, (c % 2) * CHUNK:
                                                       (c % 2 + 1) * CHUNK]
                        mm(yps[c][:], lhsT, src,
                           start=(j == 0), stop=(j == 3), reuse=(c > 0))
                for c in range(NC):
                    csl = slice(c * CHUNK, (c + 1) * CHUNK)
                    ysb = small_pool.tile([128, CHUNK], BF16, tag="ysb",
                                          bufs=4)
                    nc.vector.tensor_tensor(ysb[:], yps[c][:],
                                            yacc[mt][:, csl], op=ADD)
                    nc.sync.dma_start(yT.ap()[msl, csl], ysb[:])

    _dedupe_ldweights(nc)
    nc.finalize()
    return nc


# psum-tile name prefixes whose accumulation chains are safe to merge:
# nothing waits on their interior matmuls' semaphore values except
# long-delay buffer-reuse edges (wo prefetch, ag3 staging loads).
_MERGE_PREFIXES = ("ac_", "vps", "aq", "ak", "yp", "z")


def _merge_chain_updates(nc):
    """Move interior accumulation-chain matmul semaphore increments onto the
    chain's stop matmul (one sem-add-imm instead of N serialized sem-incs).
    The engine pays ~26ns per semaphore write; chains are 12-16 matmuls."""
    for f in nc.m.functions:
        for b in f.blocks:
            cur = {}
            chains = []
            for i in b.instructions:
                if not isinstance(i, mybir.InstMatmult):
                    continue
                out = i.outs[0]
                ref = getattr(out, "memref", None)
                if ref is None or not ref.startswith(_MERGE_PREFIXES):
                    continue
                start = bool(i.start_tensor_calc)
                stop = bool(i.stop_tensor_calc)
                if start and stop:
                    continue
                if start:
                    cur[ref] = [i]
                elif ref in cur:
                    cur[ref].append(i)
                    if stop:
                        chains.append(cur.pop(ref))
            for mem in chains:
                stop_i = mem[-1]
                ssi = stop_i.sync_info
                if ssi is None or len(ssi.on_update) != 1:
                    continue
                su = ssi.on_update[0]
                if su.sync_type != "semaphore" or su.update_mode not in (
                        "sem-inc", "sem-add-imm"):
                    continue
                moved = 0
                for i in mem[:-1]:
                    si = i.sync_info
                    if si is None or len(si.on_update) != 1:
                        continue
                    u = si.on_update[0]
                    if (u.sync_type == "semaphore" and u.id == su.id
                            and u.update_mode == "sem-inc"
                            and u.update_value == 1):
                        i.sync_info = mybir.SyncInfo(
                            on_wait=list(si.on_wait), on_update=[])
                        moved += 1
                if moved:
                    ssi.on_update = [mybir.SyncUpdate(
                        sync_type=su.sync_type, id=su.id,
                        ant_name=su.ant_name, update_mode="sem-add-imm",
                        update_value=su.update_value + moved,
                        update_reg=su.update_reg)]
                    stop_i.sync_info = ssi


def _ldw_sig(i):
    ap = i.ins[0]
    try:
        return (ap.memref, ap.offset, str(ap.ap), str(ap.dtype),
                str(i.perf_mode), str(i.tile_position), str(i.tile_size),
                bool(i.is_transpose))
    except Exception:
        return None


def _dedupe_ldweights(nc):
    """Drop InstLdweights that reload the exact weights already resident in
    the PE array (emitted per-matmul by the framework even when consecutive
    matmuls share the stationary operand). Only sync-free repeats are
    removed, so all semaphore waits/updates are preserved."""
    for f in nc.m.functions:
        for b in f.blocks:
            cur = None
            keep = []
            for i in b.instructions:
                if isinstance(i, mybir.InstLdweights):
                    sig = _ldw_sig(i)
                    si = i.sync_info
                    clean = si is None or (len(si.on_wait) == 0
                                           and len(si.on_update) == 0)
                    if sig is not None and sig == cur and clean:
                        continue
                    cur = sig
                elif isinstance(i, mybir.InstMatmult):
                    if i.is_transpose:
                        cur = None
                keep.append(i)
            if len(keep) != len(b.instructions):
                b.instructions = keep


# inner-dim permutation for per-head AllGather order:
# block (g, j) of gathered = rank j's local head g = global inner
# [(4*j + g)*128 : (4*j + g + 1)*128]
_WO_PERM = np.concatenate(
    [np.arange(128) + (4 * j + g) * 128 for g in range(4) for j in range(4)])


def _host_prep(x, rope_emb, w_q, w_k, w_v, w_o, q_norm_w, k_norm_w):
    """Build the 8 per-core input maps."""
    f = rope_emb[:, 0].astype(np.float32)  # [S, 64, 2, 2]

    def coeffs(w):
        ca = np.empty((HD, S), np.float32)
        cb = np.empty((HD, S), np.float32)
        ca[0:64] = f[:, :, 0, 0].T * w[0:64, None]
        ca[64:128] = f[:, :, 1, 1].T * w[64:128, None]
        cb[0:64] = f[:, :, 0, 1].T * w[64:128, None]
        cb[64:128] = f[:, :, 1, 0].T * w[0:64, None]
        return ca.astype(BF), cb.astype(BF)

    caq, cbq = coeffs(q_norm_w.astype(np.float32))
    cak, cbk = coeffs(k_norm_w.astype(np.float32))

    def pack_wh(w_slice):
        # w_slice: [IL, DM], head-major rows. Output [128, HL*ND*128]
        # with value[p, h*2048 + d*128 + c] = w_slice[h*128+c, d*128+p].
        wt = w_slice.T.astype(np.float32)          # [DM, IL]
        out = np.empty((128, HL * ND * 128), np.float32)
        for hh in range(HL):
            blk = wt[:, hh * 128:(hh + 1) * 128]   # [DM, 128]
            blk = blk.reshape(ND, 128, 128).transpose(1, 0, 2)
            out[:, hh * ND * 128:(hh + 1) * ND * 128] = blk.reshape(128, -1)
        return out.astype(BF)

    in_maps = []
    for c in range(8):
        b, hg = c // 4, c % 4
        sl = slice(IL * hg, IL * (hg + 1))
        in_maps.append({
            "xT": np.ascontiguousarray(x[b].T).astype(BF),
            "whq": pack_wh(w_q[sl]),
            "whk": pack_wh(w_k[sl]),
            "wvT": np.ascontiguousarray(w_v[sl].T).astype(BF),
            "woT": np.ascontiguousarray(w_o[sl][:, _WO_PERM].T).astype(BF),
            "caq": caq, "cbq": cbq, "cak": cak, "cbk": cbk,
        })
    return in_maps


def kernel(x, rope_emb, w_q, w_k, w_v, w_o, q_norm_w, k_norm_w, trace=False):
    x = np.asarray(x, dtype=np.float32)
    rope_emb = np.asarray(rope_emb, dtype=np.float32)
    w_q = np.asarray(w_q, dtype=np.float32)
    w_k = np.asarray(w_k, dtype=np.float32)
    w_v = np.asarray(w_v, dtype=np.float32)
    w_o = np.asarray(w_o, dtype=np.float32)
    q_norm_w = np.asarray(q_norm_w, dtype=np.float32)
    k_norm_w = np.asarray(k_norm_w, dtype=np.float32)
    if "nc" not in _CACHED:
        _CACHED["nc"] = _build_nc()
    nc = _CACHED["nc"]
    in_maps = _host_prep(x, rope_emb, w_q, w_k, w_v, w_o, q_norm_w, k_norm_w)
    res = run_bass_kernel_spmd(nc, in_maps, core_ids=list(range(8)),
                               trace=trace)
    _CACHED["last_result"] = res
    y = np.empty((B, S, DM), np.float32)
    for c in range(8):
        b, hg = c // 4, c % 4
        y[b, :, IL * hg:IL * (hg + 1)] = res.results[c]["yT"].T.astype(np.float32)
    return y
